# revision 41
# baseline (speedup 1.0000x reference)
"""Trainium2 Bass kernel for MAGNN link prediction (nn_MAGNN_lp).

Sharding: the B=8192 targets are split across 8 cores (1024 each) and each
core owns the metapath instances whose target falls in its range, so the
segment softmax/sum is core-local. Node towers are sharded by node rows
(5000/core, padded to 5120); the projected node table is AllGathered in
fp16 and stored as PAIRED rows [20480, 128] so every dma_gather element is
the 256B hardware minimum with no lo/hi split (idx = row>>1 fits int16;
instances are class-sorted by the parity triple of their 3 node rows so the
64-column slice offset of each tile run is compile-time).

Per chunk each core gathers all 3 metapath positions in ONE call, computes
(on fp16 DVE ops) u = ed0+ed2, the attention logit via the rotation pushed
into the attention vector (e = u.a + ed1.rot_adj(a)), exp(e-6) (global exp
shift; cancels in the softmax), and dma_scatter_adds the 136-float payload
[w*u | w*ed1 | w] into a per-target DRAM accumulator (trailing trash row
absorbs padding instances). The per-target rotation of sum(w*ed1) is applied
once per target in the head (rotation is linear), followed by normalize,
ELU, semantic attention (one tiny AllReduce), the product MLP and softmax.
Host work is integer packing of index tensors only.
"""
import numpy as np

import concourse.bass as bass
import concourse.mybir as mybir
import concourse.tile as tile
from concourse import bacc
from concourse.masks import make_identity
from dataclasses import dataclass

F32 = mybir.dt.float32
F16 = mybir.dt.float16
I16 = mybir.dt.int16
I32 = mybir.dt.int32
AF = mybir.ActivationFunctionType
ALU = mybir.AluOpType
PSUM = "PSUM"

ESHIFT = 6.0           # exp(e - ESHIFT); cancels in softmax ratio
EPS_S = 1e-9 * float(np.exp(-ESHIFT))


@dataclass
class Cfg:
    n_cores: int = 8
    B: int = 8192
    HID: int = 64
    H: int = 8
    D: int = 8
    F0: int = 512
    AV: int = 128
    CH: int = 128
    nodes_real: int = 5000      # real nodes per core
    nodes_core: int = 5120      # padded to 128
    Tc: int = 32                # max tiles per gather/scatter call
    S: int = 8                  # accumulator slots per target (HW scatter-add
                                # loses colliding updates; slots make every
                                # row within one call unique)
    T: int = 200                # tiles per metapath per core (plan sets)
    n_mp: int = 4
    gelu: bool = True           # False: Tanh stand-in (CoreSim lacks Gelu)
    dbg: bool = False
    tiles_bc: np.ndarray | None = None   # [n_bands, 8] maxed over mp, core

    @property
    def B_loc(self):
        return self.B // self.n_cores

    @property
    def n_rows(self):
        return self.nodes_core * self.n_cores    # 40960

    @property
    def n_pairs(self):
        return self.n_rows // 2                  # 20480

    @property
    def node_tiles(self):
        return self.nodes_core // 128            # 40

    @property
    def E_loc(self):
        return self.T * 128

    @property
    def kF(self):
        return self.F0 // 128

    @property
    def b_tiles(self):
        return self.B_loc // 128

    @property
    def trash_row(self):
        return self.B_loc * self.S

    @property
    def acc_rows(self):
        return self.B_loc * self.S + 128         # trailing trash rows

    @property
    def acc_step(self):
        return 192                               # 768B row stride (256B mult)

    @property
    def calls(self):
        """[(band, tile_off_in_band, ntiles)] — call windows, band-aligned."""
        out = []
        for b in range(self.tiles_bc.shape[0]):
            nb = int(self.tiles_bc[b].sum())
            off = 0
            while off < nb:
                take = min(self.Tc, nb - off)
                out.append((b, off, take))
                off += take
        return out


PAYW = 136     # payload floats per instance: w*u(64) | w*ed1(64) | w(8)


def _call_runs(tiles_bc, calls):
    """runs per call: [(toff_in_call, ntiles, cls)] from band class layout."""
    out = []
    for (b, off, nt) in calls:
        runs, t = [], 0
        for cls in range(8):
            n = int(tiles_bc[b][cls])
            a0, a1 = max(off, t), min(off + nt, t + n)
            if a0 < a1:
                runs.append((a0 - off, a1 - a0, cls))
            t += n
        out.append(runs)
    return out


def _ap_with(ap, offset_delta, tail_dims):
    """Copy an AP, keeping its partition dim, replacing trailing free dims."""
    return bass.AP(ap.tensor, ap.offset + offset_delta,
                   [list(ap.ap[0])] + [list(d) for d in tail_dims])


def build_program(cfg: Cfg):
    c = cfg
    assert c.tiles_bc is not None
    nc = bacc.Bacc("TRN2", target_bir_lowering=False, debug=False,
                   num_devices=c.n_cores)

    def di(name, shape, dtype=F32):
        return nc.dram_tensor(name, list(shape), dtype, kind="ExternalInput")

    T8 = c.T * 8
    feats = di("feats", (c.nodes_core, c.F0))
    pw = di("pw", (c.F0, c.HID))
    pb = di("pb", (c.HID,))
    w2 = di("w2", (c.HID, c.HID))
    b2 = di("b2", (c.HID,))
    g = di("g", (c.HID,))
    be = di("be", (c.HID,))
    rvec = di("rvec", (c.HID,))
    attn = di("attn", (c.n_mp, c.HID))
    emi16 = di("emi16", (c.n_mp * 128, 3 * T8), I16)
    tl16 = di("tl16", (c.n_mp * 128, T8), I16)
    suw1 = di("suw1", (c.HID, c.AV))
    sub1 = di("sub1", (c.AV,))
    suw2 = di("suw2", (c.AV,))
    siw1 = di("siw1", (c.HID, c.AV))
    sib1 = di("sib1", (c.AV,))
    siw2 = di("siw2", (c.AV,))
    cw1 = di("cw1", (c.HID, c.CH))
    cb1 = di("cb1", (c.CH,))
    cw2 = di("cw2", (c.CH, 2))
    outd = nc.dram_tensor("out", [c.B_loc, 2], F32, kind="ExternalOutput")
    if c.dbg:
        dbg_acc = nc.dram_tensor("dbg_acc", [c.B_loc, PAYW], F32,
                                 kind="ExternalOutput")
        dbg_tab = nc.dram_tensor("dbg_tab", [2048, 64], F16,
                                 kind="ExternalOutput")
        dbg_ed = nc.dram_tensor("dbg_ed", [128, 384], F16,
                                kind="ExternalOutput")

    HID, H, D, Tc = c.HID, c.H, c.D, c.Tc

    with tile.TileContext(nc) as tc:
        with (
            tc.tile_pool(name="const", bufs=1) as kpool,
            tc.tile_pool(name="dram", bufs=1, space="DRAM") as dpool,
        ):
            pk_ctx = tc.tile_pool(name="ps_const", bufs=1, space="PSUM")
            pkpool = pk_ctx.__enter__()
            # ---------- constants ----------
            id128 = kpool.tile([128, 128], F32, tag="id128")
            make_identity(nc, id128[:])
            ones1 = kpool.tile([1, 128], F32, tag="ones1")
            nc.vector.memset(ones1[:], 1.0)
            onescol = kpool.tile([128, 1], F32, tag="onescol")
            nc.vector.memset(onescol[:], 1.0)
            epscol = kpool.tile([128, 1], F32, tag="epscol")
            nc.vector.memset(epscol[:], 1e-5)
            shiftcol = kpool.tile([128, 1], F32, tag="shiftcol")
            nc.vector.memset(shiftcol[:], -ESHIFT)
            zacc = kpool.tile([128, 1560], F32, tag="zacc")
            nc.vector.memset(zacc[:], 0.0)

            def rep_row(dram_vec, n, scale=None, tag=None, dtype=F32):
                row = kpool.tile([1, n], F32, tag=f"{tag}_row")
                nc.sync.dma_start(row[:], dram_vec)
                return rep_from_row(row[:], n, tag, scale=scale, dtype=dtype)

            def rep_from_row(row_ap, n, tag, scale=None, dtype=F32):
                ps = pkpool.tile([128, 512], F32, space=PSUM, tag="reppsum")
                nc.tensor.matmul(out=ps[:, :n], lhsT=ones1[:], rhs=row_ap,
                                 start=True, stop=True)
                rep = kpool.tile([128, n], dtype, tag=tag)
                if scale is None:
                    nc.vector.tensor_copy(rep[:], ps[:, :n])
                else:
                    nc.vector.tensor_scalar_mul(rep[:], ps[:, :n], scale)
                return rep

            def vrow(x):
                return x.ap().rearrange("(o a) -> o a", o=1)

            PBrep = rep_row(vrow(pb), HID, tag="PBrep")
            B2rep = rep_row(vrow(b2), HID, tag="B2rep")
            G3rep = rep_row(vrow(g), HID, scale=1.0 / 3.0, tag="G3rep")
            BE3rep = rep_row(vrow(be), HID, scale=1.0 / 3.0, tag="BE3rep")
            SUB1rep = rep_row(vrow(sub1), c.AV, tag="SUB1rep")
            SIB1rep = rep_row(vrow(sib1), c.AV, tag="SIB1rep")
            SUW2rep = rep_row(vrow(suw2), c.AV, tag="SUW2rep")
            SIW2rep = rep_row(vrow(siw2), c.AV, tag="SIW2rep")
            CB1rep = rep_row(vrow(cb1), c.CH, tag="CB1rep")
            CW20rep = rep_row(cw2.ap()[:, 0:1].rearrange("a o -> o a"), c.CH, tag="CW20rep")
            CW21rep = rep_row(cw2.ap()[:, 1:2].rearrange("a o -> o a"), c.CH, tag="CW21rep")

            # ---------- rotation constants (normalize r on device) ----------
            # (same construction as before: crrow = Re(r) per feature,
            # c2urow/c2irow = +/- Im(r) with per-parity sign)
            rcol = kpool.tile([HID, 1], F32, tag="rcol")
            nc.sync.dma_start(rcol[:], rvec.ap().rearrange("(p o) -> p o", o=1))
            idh = kpool.tile([HID, HID], F32, tag="idh")
            make_identity(nc, idh[:])
            Sp = kpool.tile([HID, HID], F32, tag="Sp")
            nc.vector.memset(Sp[:], 0.0)
            nc.vector.tensor_copy(Sp[:, 1:HID], idh[:, 0:HID - 1])
            Sm = kpool.tile([HID, HID], F32, tag="Sm")
            nc.vector.memset(Sm[:], 0.0)
            nc.vector.tensor_copy(Sm[:, 0:HID - 1], idh[:, 1:HID])
            pidx = kpool.tile([HID, 1], I32, tag="pidx")
            nc.gpsimd.iota(pidx[:], pattern=[[0, 1]], base=0, channel_multiplier=1)
            podd_i = kpool.tile([HID, 1], I32, tag="podd_i")
            nc.vector.tensor_scalar(podd_i[:], pidx[:], 1, None, ALU.bitwise_and)
            podd = kpool.tile([HID, 1], F32, tag="podd")
            nc.vector.tensor_copy(podd[:], podd_i[:])
            peven = kpool.tile([HID, 1], F32, tag="peven")
            nc.vector.tensor_scalar(peven[:], podd[:], -1.0, -1.0, ALU.add, ALU.mult)
            Spe = kpool.tile([HID, HID], F32, tag="Spe")
            nc.vector.tensor_scalar_mul(Spe[:], Sp[:], peven[:])
            Smo = kpool.tile([HID, HID], F32, tag="Smo")
            nc.vector.tensor_scalar_mul(Smo[:], Sm[:], podd[:])
            Ie = kpool.tile([HID, HID], F32, tag="Ie")
            nc.vector.tensor_scalar_mul(Ie[:], idh[:], peven[:])
            Io = kpool.tile([HID, HID], F32, tag="Io")
            nc.vector.tensor_scalar_mul(Io[:], idh[:], podd[:])
            M2 = kpool.tile([HID, HID], F32, tag="M2")
            nc.vector.tensor_tensor(M2[:], idh[:], Spe[:], ALU.add)
            nc.vector.tensor_tensor(M2[:], M2[:], Smo[:], ALU.add)
            Me = kpool.tile([HID, HID], F32, tag="Me")
            nc.vector.tensor_tensor(Me[:], Ie[:], Spe[:], ALU.add)
            Mo = kpool.tile([HID, HID], F32, tag="Mo")
            nc.vector.tensor_tensor(Mo[:], Io[:], Smo[:], ALU.add)
            sqc = kpool.tile([HID, 1], F32, tag="sqc")
            nc.vector.tensor_tensor(sqc[:], rcol[:], rcol[:], ALU.mult)
            n2 = pkpool.tile([HID, 1], F32, space=PSUM, tag="n2")
            nc.tensor.matmul(out=n2[:], lhsT=M2[:], rhs=sqc[:], start=True, stop=True)
            nrm = kpool.tile([HID, 1], F32, tag="nrm")
            nc.scalar.activation(nrm[:], n2[:], AF.Sqrt)
            invn = kpool.tile([HID, 1], F32, tag="invn")
            nc.vector.reciprocal(invn[:], nrm[:])
            rn = kpool.tile([HID, 1], F32, tag="rn")
            nc.vector.tensor_scalar_mul(rn[:], rcol[:], invn[:])
            cr2 = pkpool.tile([HID, 1], F32, space=PSUM, tag="cr2")
            nc.tensor.matmul(out=cr2[:], lhsT=Me[:], rhs=rn[:], start=True, stop=True)
            ci2 = pkpool.tile([HID, 1], F32, space=PSUM, tag="ci2")
            nc.tensor.matmul(out=ci2[:], lhsT=Mo[:], rhs=rn[:], start=True, stop=True)
            cr2s = kpool.tile([HID, 1], F32, tag="cr2s")
            nc.vector.tensor_copy(cr2s[:], cr2[:])
            ci2s = kpool.tile([HID, 1], F32, tag="ci2s")
            nc.vector.tensor_copy(ci2s[:], ci2[:])
            crrow_ps = pkpool.tile([1, HID], F32, space=PSUM, tag="crrow_ps")
            nc.tensor.matmul(out=crrow_ps[:], lhsT=cr2s[:], rhs=idh[:], start=True, stop=True)
            crrow = kpool.tile([1, HID], F32, tag="crrow")
            nc.vector.tensor_copy(crrow[:], crrow_ps[:])
            cirow_ps = pkpool.tile([1, HID], F32, space=PSUM, tag="cirow_ps")
            nc.tensor.matmul(out=cirow_ps[:], lhsT=ci2s[:], rhs=idh[:], start=True, stop=True)
            cirow = kpool.tile([1, HID], F32, tag="cirow")
            nc.vector.tensor_copy(cirow[:], cirow_ps[:])
            fidx = kpool.tile([1, HID], I32, tag="fidx")
            nc.gpsimd.iota(fidx[:], pattern=[[1, HID]], base=0, channel_multiplier=0)
            fodd_i = kpool.tile([1, HID], I32, tag="fodd_i")
            nc.vector.tensor_scalar(fodd_i[:], fidx[:], 1, None, ALU.bitwise_and)
            fsign = kpool.tile([1, HID], F32, tag="fsign")
            nc.vector.tensor_copy(fsign[:], fodd_i[:])
            nc.vector.tensor_scalar(fsign[:], fsign[:], -2.0, 1.0, ALU.mult, ALU.add)
            c2u = kpool.tile([1, HID], F32, tag="c2u")
            c2i = kpool.tile([1, HID], F32, tag="c2i")
            c2row = [c2u, c2i]
            nc.vector.tensor_tensor(c2row[0][:], cirow[:], fsign[:], ALU.mult)
            nc.vector.tensor_scalar_mul(c2row[1][:], c2row[0][:], -1.0)
            C1rep = rep_from_row(crrow[:], HID, "C1rep")
            C2rep = [rep_from_row(c2row[0][:], HID, "C2urep"),
                     rep_from_row(c2row[1][:], HID, "C2irep")]

            # attention rows (fp16 replicated) + rotation-adjoint rows:
            # a2_d = C1_d * a_d + C2_{d^1} * a_{d^1}
            ATT16, A2_16 = [], []
            for mp in range(c.n_mp):
                side = 0 if mp < 2 else 1
                arow = kpool.tile([1, HID], F32, tag=f"arow{mp}")
                nc.sync.dma_start(arow[:], attn.ap()[mp:mp + 1, :])
                m = kpool.tile([1, HID], F32, tag=f"m{mp}")
                nc.vector.tensor_tensor(m[:], c2row[side][:], arow[:], ALU.mult)
                a2 = kpool.tile([1, HID], F32, tag=f"a2_{mp}")
                nc.vector.tensor_tensor(a2[:], crrow[:], arow[:], ALU.mult)
                mswap = _ap_with(m[:], 1, [[2, HID // 2], [-1, 2]])
                nc.vector.tensor_tensor(a2[:], a2[:], mswap, ALU.add)
                ATT16.append(rep_from_row(arow[:], HID, f"AT16_{mp}", dtype=F16))
                A2_16.append(rep_from_row(a2[:], HID, f"A216_{mp}", dtype=F16))

            pwsb = kpool.tile([128, c.kF, HID], F32, tag="pwsb")
            nc.sync.dma_start(pwsb[:], pw.ap().rearrange("(a p) c -> p a c", p=128))
            w2sb = kpool.tile([HID, HID], F32, tag="w2sb")
            nc.sync.dma_start(w2sb[:], w2.ap())
            suw1sb = kpool.tile([HID, c.AV], F32, tag="suw1sb")
            nc.sync.dma_start(suw1sb[:], suw1.ap())
            siw1sb = kpool.tile([HID, c.AV], F32, tag="siw1sb")
            nc.sync.dma_start(siw1sb[:], siw1.ap())
            cw1sb = kpool.tile([HID, c.CH], F32, tag="cw1sb")
            nc.sync.dma_start(cw1sb[:], cw1.ap())

            pk_ctx.__exit__(None, None, None)

            # ---------- dram tiles ----------
            tower_t = dpool.tile([c.nodes_core, HID], F16, tag="tower")
            table_t = dpool.tile([c.n_rows, HID], F16, tag="table")
            accs = [dpool.tile([c.acc_rows, c.acc_step], F32, tag=f"acc{mp}",
                               name=f"acc{mp}")
                    for mp in range(c.n_mp)]
            # acc_rows*acc_step == 128*12480; zero in 8 contiguous strips
            assert c.acc_rows * c.acc_step == 128 * 12480
            for mp in range(c.n_mp):
                for j in range(8):
                    dst = bass.AP(accs[mp][:].tensor,
                                  accs[mp][:].offset + j * 1560,
                                  [[12480, 128], [1, 1560]])
                    nc.sync.dma_start(dst, zacc[:])

            # ---------- tower (two passes to avoid act-table thrash) ----------
            nt = c.node_tiles
            with (
                tc.tile_pool(name="tw_x", bufs=2) as xpool,
                tc.tile_pool(name="tw_ps", bufs=2, space="PSUM") as tpspool,
                tc.tile_pool(name="tw_s", bufs=2) as tspool,
                tc.tile_pool(name="tw_keep", bufs=1) as tkpool,
            ):
                Z = tkpool.tile([128, nt, HID], F32, tag="Z")
                HH = tkpool.tile([128, nt, HID], F32, tag="HH")
                YC = tkpool.tile([128, nt, HID], F32, tag="YC")
                VV = tkpool.tile([128, nt], F32, tag="VV")
                for j in range(nt):
                    xt = xpool.tile([128, c.F0], F32, tag="xt")
                    nc.sync.dma_start(xt[:], feats.ap()[j * 128:(j + 1) * 128, :])
                    xT = xpool.tile([128, c.kF, 128], F32, tag="xT")
                    for kk in range(c.kF):
                        pst = tpspool.tile([128, 128], F32, space=PSUM, tag="pst")
                        nc.tensor.transpose(pst[:], xt[:, kk * 128:(kk + 1) * 128], id128[:])
                        nc.vector.tensor_copy(xT[:, kk, :], pst[:])
                    z = tpspool.tile([128, HID], F32, space=PSUM, tag="z")
                    for kk in range(c.kF):
                        nc.tensor.matmul(out=z[:], lhsT=xT[:, kk, :], rhs=pwsb[:, kk, :],
                                         start=(kk == 0), stop=(kk == c.kF - 1))
                    nc.vector.tensor_tensor(Z[:, j, :], z[:], PBrep[:], ALU.add)
                # one activation call for the whole tower
                nc.scalar.activation(HH[:].rearrange("p a b -> p (a b)"),
                                     Z[:].rearrange("p a b -> p (a b)"),
                                     AF.Gelu if c.gelu else AF.Tanh)
                for j in range(nt):
                    hT_ps = tpspool.tile([HID, 128], F32, space=PSUM, tag="hT_ps")
                    nc.tensor.transpose(hT_ps[:], HH[:, j, :], id128[:])
                    hT = tspool.tile([HID, 128], F32, tag="hT")
                    nc.vector.tensor_copy(hT[:], hT_ps[:])
                    y = tpspool.tile([128, HID], F32, space=PSUM, tag="y")
                    nc.tensor.matmul(out=y[:], lhsT=hT[:], rhs=w2sb[:], start=True, stop=True)
                    ys = tspool.tile([128, HID], F32, tag="ys")
                    nc.vector.tensor_tensor(ys[:], y[:], B2rep[:], ALU.add)
                    nc.vector.tensor_tensor(ys[:], ys[:], Z[:, j, :], ALU.add)
                    mu = tspool.tile([128, 1], F32, tag="mu")
                    nc.vector.tensor_reduce(mu[:], ys[:], mybir.AxisListType.X, ALU.add)
                    nc.vector.tensor_scalar_mul(mu[:], mu[:], 1.0 / HID)
                    nc.vector.tensor_scalar(YC[:, j, :], ys[:], mu[:], None, ALU.subtract)
                    sq = tspool.tile([128, HID], F32, tag="sq")
                    nc.vector.tensor_tensor(sq[:], YC[:, j, :], YC[:, j, :], ALU.mult)
                    nc.vector.tensor_reduce(VV[:, j:j + 1], sq[:], mybir.AxisListType.X, ALU.add)
                sdv = tspool.tile([128, nt], F32, tag="sdv")
                nc.scalar.activation(sdv[:], VV[:], AF.Sqrt, bias=epscol[:],
                                     scale=1.0 / HID)
                inv = tspool.tile([128, nt], F32, tag="inv")
                nc.vector.reciprocal(inv[:], sdv[:])
                t1 = tkpool.tile([128, nt, HID], F32, tag="t1")
                invb = _ap_with(inv[:], 0, [[1, nt], [0, HID]])
                nc.vector.tensor_tensor(t1[:], YC[:], invb, ALU.mult)
                g3b = _ap_with(G3rep[:], 0, [[0, nt], [1, HID]])
                be3b = _ap_with(BE3rep[:], 0, [[0, nt], [1, HID]])
                TS = tkpool.tile([128, nt, HID], F16, tag="TS")
                nc.vector.tensor_tensor(TS[:], t1[:], g3b, ALU.mult)
                nc.vector.tensor_tensor(TS[:], TS[:], be3b, ALU.add)
                # SBUF [128, nt, HID] -> DRAM rows (j*128+p)
                dst = bass.AP(tower_t[:].tensor, tower_t[:].offset,
                              [[HID, 128], [128 * HID, nt], [1, HID]])
                nc.sync.dma_start(dst, TS[:])

            nc.gpsimd.collective_compute(
                "AllGather", ALU.bypass,
                replica_groups=[list(range(c.n_cores))],
                ins=[tower_t.opt()], outs=[table_t.opt()],
            )
            # paired-row view for gathers: [n_pairs, 128] fp16
            table_pairs = bass.AP(table_t[:].tensor, table_t[:].offset,
                                  [[128, c.n_pairs], [1, 128]])

            # ---------- metapath chunks: gather, logits, scatter-add ----------
            calls = c.calls
            runs_all = _call_runs(c.tiles_bc, calls)
            # per-call column offsets into the packed emi16 / tl16 streams
            emi_off, tl_off = [0], [0]
            for (_, _, nt) in calls:
                emi_off.append(emi_off[-1] + 3 * nt * 8)
                tl_off.append(tl_off[-1] + nt * 8)
            with (
                tc.tile_pool(name="mp_idx", bufs=2) as ipool,
                tc.tile_pool(name="mp_ed", bufs=2) as edpool,
                tc.tile_pool(name="mp_row", bufs=2) as rowpool,
                tc.tile_pool(name="mp_tmp", bufs=1) as mtpool,
                tc.tile_pool(name="mp_tl", bufs=1) as tlpool,
                tc.tile_pool(name="hd_acc", bufs=2) as apool,
                tc.tile_pool(name="hd_s", bufs=3) as hpool,
                tc.tile_pool(name="hd_ps", bufs=1, space="PSUM") as hpspool,
                tc.tile_pool(name="hd_keep", bufs=1) as keep,
            ):
                outs_all = keep.tile([128, c.n_mp, c.b_tiles, HID], F32, tag="outs_all")
                acc4 = keep.tile([1, c.n_mp], F32, tag="acc4")
                nc.vector.memset(acc4[:], 0.0)

                emi_sbs, tl_sbs, eds = {}, {}, {}

                def emit_gather(mp, ci):
                    _, _, nt = calls[ci]
                    ed = edpool.tile([128, 3 * Tc, 128], F16, tag="ed")
                    eds[(mp, ci)] = ed
                    nc.gpsimd.dma_gather(
                        out_ap=ed[:, 0:3 * nt, :], in_ap=table_pairs,
                        idxs_ap=emi_sbs[mp][:, emi_off[ci]:emi_off[ci + 1]],
                        num_idxs=3 * nt * 128, num_idxs_reg=3 * nt * 128,
                        elem_size=128, single_packet=False)

                def emit_compute_scatter(mp, ci):
                    ed = eds.pop((mp, ci))
                    _, _, nt = calls[ci]
                    runs = runs_all[ci]
                    u = mtpool.tile([128, Tc, HID], F16, tag="u")
                    v = mtpool.tile([128, Tc, HID], F16, tag="v")
                    t2 = mtpool.tile([128, Tc, HID], F16, tag="t2")
                    for (t0, ntk, cls) in runs:
                        p0, p1, p2 = cls & 1, (cls >> 1) & 1, (cls >> 2) & 1
                        e0 = ed[:, t0:t0 + ntk, p0 * HID:p0 * HID + HID]
                        e1 = ed[:, nt + t0:nt + t0 + ntk, p1 * HID:p1 * HID + HID]
                        e2 = ed[:, 2 * nt + t0:2 * nt + t0 + ntk, p2 * HID:p2 * HID + HID]
                        nc.vector.tensor_tensor(u[:, t0:t0 + ntk, :], e0, e2, ALU.add)
                        a2b = _ap_with(A2_16[mp][:], 0, [[0, ntk], [1, HID]])
                        nc.vector.tensor_tensor(t2[:, t0:t0 + ntk, :], e1, a2b, ALU.mult)
                    ab = _ap_with(ATT16[mp][:], 0, [[0, nt], [1, HID]])
                    nc.vector.tensor_tensor(v[:, 0:nt, :], u[:, 0:nt, :], ab, ALU.mult)
                    nc.vector.tensor_tensor(v[:, 0:nt, :], v[:, 0:nt, :],
                                            t2[:, 0:nt, :], ALU.add)
                    e32 = mtpool.tile([128, Tc, H], F32, tag="e32")
                    nc.vector.tensor_reduce(
                        e32[:, 0:nt, :],
                        v[:, 0:nt, :].rearrange("p t (h d) -> p t h d", d=D),
                        mybir.AxisListType.X, ALU.add)
                    el = mtpool.tile([128, Tc, H], F32, tag="el")
                    nc.vector.tensor_scalar_mul(el[:, 0:nt, :], e32[:, 0:nt, :], 0.01)
                    nc.vector.tensor_tensor(el[:, 0:nt, :], el[:, 0:nt, :],
                                            e32[:, 0:nt, :], ALU.max)
                    w16 = mtpool.tile([128, Tc, H], F16, tag="w16")
                    nc.scalar.activation(w16[:, 0:nt, :], el[:, 0:nt, :], AF.Exp,
                                         bias=shiftcol[:])
                    rows = rowpool.tile([128, Tc, PAYW], F32, tag="rows")
                    wb = _ap_with(w16[:], 0, [[H, nt], [1, H], [0, D]])
                    nc.vector.tensor_tensor(rows[:, 0:nt, 0:HID], u[:, 0:nt, :],
                                            wb, ALU.mult)
                    nc.vector.tensor_copy(rows[:, 0:nt, 2 * HID:PAYW], w16[:, 0:nt, :])
                    for (t0, ntk, cls) in runs:
                        p1 = (cls >> 1) & 1
                        e1 = ed[:, nt + t0:nt + t0 + ntk, p1 * HID:p1 * HID + HID]
                        wbr = _ap_with(w16[:], t0 * H, [[H, ntk], [1, H], [0, D]])
                        nc.vector.tensor_tensor(rows[:, t0:t0 + ntk, HID:2 * HID],
                                                e1, wbr, ALU.mult)
                    acc_ap = bass.AP(accs[mp][:].tensor, accs[mp][:].offset,
                                     [[c.acc_step, c.acc_rows], [1, PAYW]])
                    nc.gpsimd.dma_scatter_add(
                        out_ap=acc_ap, in_ap=rows[:, 0:nt, :],
                        idxs_ap=tl_sbs[mp][:, tl_off[ci]:tl_off[ci + 1]],
                        num_idxs=nt * 128, num_idxs_reg=nt * 128,
                        elem_size=PAYW, elem_step=c.acc_step,
                        single_packet=False)

                def emit_head(mp):
                    side = 0 if mp < 2 else 1
                    w1sb = suw1sb if mp < 2 else siw1sb
                    b1rep = SUB1rep if mp < 2 else SIB1rep
                    w2rep = SUW2rep if mp < 2 else SIW2rep
                    for bt in range(c.b_tiles):
                        acc_sb = apool.tile([128, c.S, PAYW], F32, tag="acc_sb")
                        src = bass.AP(accs[mp][:].tensor,
                                      accs[mp][:].offset
                                      + bt * 128 * c.S * c.acc_step,
                                      [[c.S * c.acc_step, 128],
                                       [c.acc_step, c.S], [1, PAYW]])
                        nc.sync.dma_start(acc_sb[:], src)
                        # fold the 8 slots
                        f4 = apool.tile([128, 4, PAYW], F32, tag="f4")
                        nc.vector.tensor_tensor(f4[:], acc_sb[:, 0:4, :],
                                                acc_sb[:, 4:8, :], ALU.add)
                        f2 = apool.tile([128, 2, PAYW], F32, tag="f2")
                        nc.vector.tensor_tensor(f2[:], f4[:, 0:2, :],
                                                f4[:, 2:4, :], ALU.add)
                        f1 = apool.tile([128, PAYW], F32, tag="f1")
                        nc.vector.tensor_tensor(f1[:], f2[:, 0, :],
                                                f2[:, 1, :], ALU.add)
                        s02 = f1[:, 0:HID]
                        s1 = f1[:, HID:2 * HID]
                        sw = f1[:, 2 * HID:PAYW]
                        den = hpool.tile([128, H], F32, tag="den")
                        nc.vector.tensor_scalar_add(den[:], sw, EPS_S)
                        dinv = hpool.tile([128, H], F32, tag="dinv")
                        nc.vector.reciprocal(dinv[:], den[:])
                        rot = hpool.tile([128, HID], F32, tag="rot")
                        nc.vector.tensor_tensor(rot[:], s1, C1rep[:], ALU.mult)
                        tb = hpool.tile([128, HID], F32, tag="tb")
                        s1swap = _ap_with(s1, 1, [[2, HID // 2], [-1, 2]])
                        nc.vector.tensor_tensor(tb[:], s1swap, C2rep[side][:], ALU.mult)
                        nc.vector.tensor_tensor(rot[:], rot[:], tb[:], ALU.add)
                        nc.vector.tensor_tensor(rot[:], rot[:], s02, ALU.add)
                        ret = hpool.tile([128, HID], F32, tag="ret")
                        dinvb = _ap_with(dinv[:], 0, [[1, H], [0, D]])
                        nc.vector.tensor_tensor(ret[:], rot[:], dinvb, ALU.mult)
                        neg = hpool.tile([128, HID], F32, tag="neg")
                        nc.vector.tensor_scalar_min(neg[:], ret[:], 0.0)
                        en = hpool.tile([128, HID], F32, tag="en")
                        nc.scalar.activation(en[:], neg[:], AF.Exp)
                        o = outs_all[:, mp, bt, :]
                        nc.vector.tensor_scalar_max(ret[:], ret[:], 0.0)
                        nc.vector.tensor_scalar_add(en[:], en[:], -1.0)
                        nc.vector.tensor_tensor(o, ret[:], en[:], ALU.add)
                    for bt in range(c.b_tiles):
                        o = outs_all[:, mp, bt, :]
                        oT_ps = hpspool.tile([HID, 128], F32, space=PSUM, tag="oT_ps")
                        nc.tensor.transpose(oT_ps[:], o, id128[:])
                        oT = hpool.tile([HID, 128], F32, tag="oT")
                        nc.vector.tensor_copy(oT[:], oT_ps[:])
                        tt = hpspool.tile([128, c.AV], F32, space=PSUM, tag="tt")
                        nc.tensor.matmul(out=tt[:], lhsT=oT[:], rhs=w1sb[:], start=True, stop=True)
                        th = hpool.tile([128, c.AV], F32, tag="th")
                        nc.vector.tensor_tensor(th[:], tt[:], b1rep[:], ALU.add)
                        nc.scalar.activation(th[:], th[:], AF.Tanh)
                        nc.vector.tensor_tensor(th[:], th[:], w2rep[:], ALU.mult)
                        rsum = hpool.tile([128, 1], F32, tag="rsum")
                        nc.vector.tensor_reduce(rsum[:], th[:], mybir.AxisListType.X, ALU.add)
                        sp = hpspool.tile([1, 1], F32, space=PSUM, tag="sp")
                        nc.tensor.matmul(out=sp[:], lhsT=rsum[:], rhs=onescol[:], start=True, stop=True)
                        nc.vector.tensor_tensor(acc4[:, mp:mp + 1], acc4[:, mp:mp + 1], sp[:], ALU.add)

                # software-pipelined emission: gather(ci+1) before compute(ci)
                for mp in range(c.n_mp):
                    emi_sb = ipool.tile([128, 3 * T8], I16, tag="emi_sb")
                    nc.sync.dma_start(
                        emi_sb[:], emi16.ap()[mp * 128:(mp + 1) * 128, :])
                    emi_sbs[mp] = emi_sb
                    tl_sb = tlpool.tile([128, T8], I16, tag="tl_sb")
                    nc.sync.dma_start(tl_sb[:], tl16.ap()[mp * 128:(mp + 1) * 128, :])
                    tl_sbs[mp] = tl_sb
                    for ci in range(len(calls)):
                        emit_gather(mp, ci)
                        if ci > 0:
                            emit_compute_scatter(mp, ci - 1)
                    emit_compute_scatter(mp, len(calls) - 1)
                    emit_head(mp)

                if c.dbg:
                    dbg_ctx = tc.tile_pool(name="dbgp", bufs=1)
                    dpool_dbg = dbg_ctx.__enter__()
                    for b0 in range(0, c.b_tiles, 2):
                        dsb = dpool_dbg.tile([128, 2, PAYW], F32, tag="dsb",
                                             name=f"dsb{b0}")
                        src = bass.AP(accs[0][:].tensor,
                                      accs[0][:].offset + b0 * 128 * c.acc_step,
                                      [[c.acc_step, 128], [128 * c.acc_step, 2],
                                       [1, PAYW]])
                        nc.sync.dma_start(dsb[:], src)
                        dst = bass.AP(dbg_acc.ap().tensor, b0 * 128 * PAYW,
                                      [[PAYW, 128], [128 * PAYW, 2], [1, PAYW]])
                        nc.sync.dma_start(dst, dsb[:])
                    tsb = dpool_dbg.tile([128, 16, 64], F16, tag="tsb")
                    nc.sync.dma_start(
                        tsb[:], bass.AP(table_t[:].tensor, 0,
                                        [[64, 128], [128 * 64, 16], [1, 64]]))
                    dst2 = bass.AP(dbg_tab.ap().tensor, 0,
                                   [[64, 128], [128 * 64, 16], [1, 64]])
                    nc.sync.dma_start(dst2, tsb[:])
                    esb = dpool_dbg.tile([128, 384], F16, tag="esb")
                    nc.gpsimd.dma_gather(
                        out_ap=esb[:].rearrange("p (a b) -> p a b", a=3),
                        in_ap=table_pairs,
                        idxs_ap=emi_sbs[c.n_mp - 1][:, 0:24],
                        num_idxs=384, num_idxs_reg=384,
                        elem_size=128, single_packet=False)
                    nc.sync.dma_start(dbg_ed.ap(), esb[:])
                    dbg_ctx.__exit__(None, None, None)

                # ---------- semantic softmax + product MLP ----------
                sin_t = dpool.tile([1, 128], F32, tag="sin")
                sout_t = dpool.tile([1, 128], F32, tag="sout")
                zrow = hpool.tile([1, 128], F32, tag="zrow")
                nc.vector.memset(zrow[:], 0.0)
                nc.sync.dma_start(sin_t[:], zrow[:])
                nc.sync.dma_start(sin_t[0:1, 0:c.n_mp], acc4[:])
                nc.gpsimd.collective_compute(
                    "AllReduce", ALU.add,
                    replica_groups=[list(range(c.n_cores))],
                    ins=[sin_t.opt()], outs=[sout_t.opt()],
                )
                s4 = hpool.tile([1, c.n_mp], F32, tag="s4")
                nc.sync.dma_start(s4[:], sout_t[0:1, 0:c.n_mp])
                e4 = hpool.tile([1, c.n_mp], F32, tag="e4")
                nc.scalar.activation(e4[:], s4[:], AF.Exp, scale=1.0 / c.B)
                beta = hpool.tile([1, c.n_mp], F32, tag="beta")
                for sd in range(2):
                    ssum = hpool.tile([1, 1], F32, tag="ssum")
                    nc.vector.tensor_reduce(ssum[:], e4[:, 2 * sd:2 * sd + 2],
                                            mybir.AxisListType.X, ALU.add)
                    sinv = hpool.tile([1, 1], F32, tag="sinv")
                    nc.vector.reciprocal(sinv[:], ssum[:])
                    nc.vector.tensor_scalar_mul(beta[:, 2 * sd:2 * sd + 2],
                                                e4[:, 2 * sd:2 * sd + 2], sinv[:])
                bc_ps = hpspool.tile([128, c.n_mp], F32, space=PSUM, tag="bc_ps")
                nc.tensor.matmul(out=bc_ps[:], lhsT=ones1[:], rhs=beta[:], start=True, stop=True)
                bcol = keep.tile([128, c.n_mp], F32, tag="bcol")
                nc.vector.tensor_copy(bcol[:], bc_ps[:])

                for bt in range(c.b_tiles):
                    hu = hpool.tile([128, HID], F32, tag="hu")
                    hi_ = hpool.tile([128, HID], F32, tag="hi_")
                    t0 = hpool.tile([128, HID], F32, tag="t0")
                    nc.vector.tensor_scalar_mul(hu[:], outs_all[:, 0, bt, :], bcol[:, 0:1])
                    nc.vector.tensor_scalar_mul(t0[:], outs_all[:, 1, bt, :], bcol[:, 1:2])
                    nc.vector.tensor_tensor(hu[:], hu[:], t0[:], ALU.add)
                    nc.vector.tensor_scalar_mul(hi_[:], outs_all[:, 2, bt, :], bcol[:, 2:3])
                    nc.vector.tensor_scalar_mul(t0[:], outs_all[:, 3, bt, :], bcol[:, 3:4])
                    nc.vector.tensor_tensor(hi_[:], hi_[:], t0[:], ALU.add)
                    xx = hpool.tile([128, HID], F32, tag="xx")
                    nc.vector.tensor_tensor(xx[:], hu[:], hi_[:], ALU.mult)
                    xT_ps = hpspool.tile([HID, 128], F32, space=PSUM, tag="xT_ps")
                    nc.tensor.transpose(xT_ps[:], xx[:], id128[:])
                    xT = hpool.tile([HID, 128], F32, tag="xT")
                    nc.vector.tensor_copy(xT[:], xT_ps[:])
                    yy = hpspool.tile([128, c.CH], F32, space=PSUM, tag="yy")
                    nc.tensor.matmul(out=yy[:], lhsT=xT[:], rhs=cw1sb[:], start=True, stop=True)
                    ya = hpool.tile([128, c.CH], F32, tag="ya")
                    nc.vector.tensor_tensor(ya[:], yy[:], CB1rep[:], ALU.add)
                    nc.scalar.activation(ya[:], ya[:], AF.Relu)
                    l0t = hpool.tile([128, c.CH], F32, tag="l0t")
                    nc.vector.tensor_tensor(l0t[:], ya[:], CW20rep[:], ALU.mult)
                    l0 = hpool.tile([128, 1], F32, tag="l0")
                    nc.vector.tensor_reduce(l0[:], l0t[:], mybir.AxisListType.X, ALU.add)
                    nc.vector.tensor_tensor(l0t[:], ya[:], CW21rep[:], ALU.mult)
                    l1 = hpool.tile([128, 1], F32, tag="l1")
                    nc.vector.tensor_reduce(l1[:], l0t[:], mybir.AxisListType.X, ALU.add)
                    dl = hpool.tile([128, 1], F32, tag="dl")
                    ot = hpool.tile([128, 2], F32, tag="ot")
                    nc.vector.tensor_tensor(dl[:], l0[:], l1[:], ALU.subtract)
                    nc.scalar.activation(ot[:, 0:1], dl[:], AF.Sigmoid)
                    nc.vector.tensor_tensor(dl[:], l1[:], l0[:], ALU.subtract)
                    nc.scalar.activation(ot[:, 1:2], dl[:], AF.Sigmoid)
                    nc.sync.dma_start(outd.ap()[bt * 128:(bt + 1) * 128, :], ot[:])

    nc.compile()
    return nc


# ---------------------------------------------------------------------------
# host side: sharding / packing (integer work only)
# ---------------------------------------------------------------------------

def _mp_arrays(inputs, mp):
    if mp < 2:
        return np.asarray(inputs["emi_user"][mp]), np.asarray(inputs["tgt_user"][mp])
    return np.asarray(inputs["emi_item"][mp - 2]), np.asarray(inputs["tgt_item"][mp - 2])


def _rows_of(emi, c: Cfg):
    """Global node id -> padded table row id."""
    return (emi // c.nodes_real) * c.nodes_core + emi % c.nodes_real


def _band_cls(emi, tgt, k, c: Cfg):
    """Per-core (band, cls, tloc, rows) for the selected instances."""
    lo, hi = k * c.B_loc, (k + 1) * c.B_loc
    sel = np.nonzero((tgt >= lo) & (tgt < hi))[0]
    r_all = _rows_of(emi[sel], c)
    t_all = tgt[sel] - lo
    cls = (r_all[:, 0] & 1) + 2 * (r_all[:, 1] & 1) + 4 * (r_all[:, 2] & 1)
    order = np.argsort(t_all, kind="stable")
    ts = t_all[order]
    rk = np.arange(ts.size) - np.searchsorted(ts, ts, side="left")
    band = np.empty_like(rk)
    slot = np.empty_like(rk)
    band[order] = rk // c.S
    slot[order] = rk % c.S
    return band, slot, cls, t_all, r_all


def make_plan(inputs, cfg: Cfg):
    """tiles_bc [n_bands, 8]: tiles per (rank-band, parity-class), maxed over
    every (metapath, core) so one compiled layout serves all shards."""
    c = cfg
    nb = 0
    cnts = []
    for mp in range(c.n_mp):
        emi, tgt = _mp_arrays(inputs, mp)
        for k in range(c.n_cores):
            band, slot, cls, t_all, _ = _band_cls(emi, tgt, k, c)
            nb = max(nb, int(band.max()) + 1)
            cnt = np.zeros((int(band.max()) + 1, 8), np.int64)
            np.add.at(cnt, (band, cls), 1)
            cnts.append(cnt)
    tiles_bc = np.zeros((nb, 8), np.int64)
    for cnt in cnts:
        t = (cnt + 127) // 128
        tiles_bc[:t.shape[0]] = np.maximum(tiles_bc[:t.shape[0]], t)
    return tiles_bc, int(tiles_bc.sum())


def _wrap16(vals):
    """[N] values (N % 16 == 0) -> [128, N/16] int16, q7 wrapped layout."""
    v = np.asarray(vals).astype(np.int16).reshape(-1, 16)
    return np.ascontiguousarray(np.tile(v.T, (8, 1)))


def _pack_metapath(emi, tgt, k, c: Cfg):
    """Pack one (metapath, core) shard: band-major, class-sorted in band.

    Row index for the scatter is tgt*S + rank%S (unique within any call,
    since calls never span a band boundary); padding goes to the trash row.
    Returns (emi16 [128, sum(3*nt*8)], tl16 [128, T*8])."""
    band, slot, cls, t_all, r_all = _band_cls(emi, tgt, k, c)
    tiles_bc = c.tiles_bc
    E = c.T * 128
    r_sh = np.zeros((E, 3), np.int64)
    rowi = np.full((E,), c.trash_row, np.int64)
    tpos = 0
    for b in range(tiles_bc.shape[0]):
        for cl in range(8):
            ntiles = int(tiles_bc[b][cl])
            if ntiles == 0:
                continue
            seg = np.nonzero((band == b) & (cls == cl))[0]
            assert seg.size <= ntiles * 128, (b, cl, seg.size, ntiles)
            base = tpos * 128
            r_sh[base:base + seg.size] = r_all[seg]
            dummy = np.array([(cl >> l) & 1 for l in range(3)], np.int64)
            r_sh[base + seg.size:base + ntiles * 128] = dummy
            rowi[base:base + seg.size] = t_all[seg] * c.S + slot[seg]
            tpos += ntiles
    assert tpos == c.T
    pair = r_sh >> 1          # [E, 3] pair-row gather indices (< 20480)
    emi_calls, tl_calls = [], []
    t0 = 0
    for (_, _, nt) in c.calls:
        blk = slice(t0 * 128, (t0 + nt) * 128)
        stream = np.concatenate([pair[blk, l] for l in range(3)])
        emi_calls.append(_wrap16(stream))
        tl_calls.append(_wrap16(rowi[blk]))
        t0 += nt
    assert t0 == c.T
    return (np.concatenate(emi_calls, axis=1),
            np.concatenate(tl_calls, axis=1))


def prepare(inputs, cfg: Cfg):
    c = cfg
    tbc, T = make_plan(inputs, cfg)
    c.tiles_bc = tbc
    c.T = T

    f0, f1 = np.asarray(inputs["feats0"]), np.asarray(inputs["feats1"])
    feats_all = np.concatenate([f0, f1], axis=0)
    attn4 = np.stack([np.asarray(inputs["attn_user"][p]).reshape(-1) for p in range(2)] +
                     [np.asarray(inputs["attn_item"][p]).reshape(-1) for p in range(2)])
    rv = np.asarray(inputs["r_vec"])[0].reshape(-1).astype(np.float32)

    in_maps = []
    for k in range(c.n_cores):
        m = {}
        lo_n = k * c.nodes_real
        fs = feats_all[lo_n:lo_n + c.nodes_real]
        pad = c.nodes_core - c.nodes_real
        if pad:
            fs = np.concatenate([fs, np.zeros((pad, c.F0), np.float32)], axis=0)
        m["feats"] = np.ascontiguousarray(fs, np.float32)
        tw = "0" if lo_n < f0.shape[0] else "1"
        for nm in ("pw", "pb", "w2", "b2", "g", "be"):
            m[nm] = np.asarray(inputs[f"tower{tw}_{nm}"], np.float32)
        m["rvec"] = rv
        m["attn"] = attn4.astype(np.float32)
        emi_l, tl_l = [], []
        for mp in range(c.n_mp):
            emi, tgt = _mp_arrays(inputs, mp)
            e16, t16 = _pack_metapath(emi, tgt, k, c)
            emi_l.append(e16)
            tl_l.append(t16)
        m["emi16"] = np.concatenate(emi_l, axis=0)
        m["tl16"] = np.concatenate(tl_l, axis=0)
        for nm in ("su_w1", "su_b1", "su_w2", "si_w1", "si_b1", "si_w2",
                   "cw1", "cb1", "cw2"):
            m[nm.replace("_", "")] = np.asarray(inputs[nm], np.float32)
        in_maps.append(m)
    return in_maps


# ---------------------------------------------------------------------------
# PJRT SPMD runner (axon path)
# ---------------------------------------------------------------------------


class SpmdRunner:
    def __init__(self, nc, n_cores: int):
        import jax
        from jax.sharding import Mesh, PartitionSpec, NamedSharding
        from jax.experimental.shard_map import shard_map
        from concourse.bass2jax import (
            _bass_exec_p, install_neuronx_cc_hook, partition_id_tensor)

        self.jax = jax
        install_neuronx_cc_hook()
        self.nc = nc
        self.n_cores = n_cores
        partition_name = nc.partition_id_tensor.name if nc.partition_id_tensor else None
        in_names, out_names, out_avals, zero_outs = [], [], [], []
        for alloc in nc.m.functions[0].allocations:
            if not isinstance(alloc, mybir.MemoryLocationSet):
                continue
            name = alloc.memorylocations[0].name
            if alloc.kind == "ExternalInput":
                if name != partition_name:
                    in_names.append(name)
            elif alloc.kind == "ExternalOutput":
                out_names.append(name)
                shape = tuple(alloc.tensor_shape)
                dtype = mybir.dt.np(alloc.dtype)
                out_avals.append(jax.core.ShapedArray(shape, dtype))
                zero_outs.append(np.zeros(shape, dtype))
        self.dbg_name = nc.dbg_addr.name if nc.dbg_addr is not None else None
        n_params = len(in_names)
        in_names = in_names + out_names
        if partition_name is not None:
            in_names.append(partition_name)
        self.in_names, self.out_names = in_names, out_names
        self.n_params, self.out_avals, self.zero_outs = n_params, out_avals, zero_outs

        def _body(*args):
            operands = list(args)
            if partition_name is not None:
                operands.append(partition_id_tensor())
            outs = _bass_exec_p.bind(
                *operands,
                out_avals=tuple(out_avals),
                in_names=tuple(in_names),
                out_names=tuple(out_names),
                lowering_input_output_aliases=(),
                sim_require_finite=True,
                sim_require_nnan=True,
                nc=nc,
            )
            return tuple(outs)

        devices = jax.devices()[:n_cores]
        assert len(devices) == n_cores
        self.mesh = Mesh(np.asarray(devices), ("core",))
        donate = tuple(range(n_params, n_params + len(out_names)))
        in_specs = (PartitionSpec("core"),) * (n_params + len(out_names))
        out_specs = (PartitionSpec("core"),) * len(out_names)
        self.sharded = jax.jit(
            shard_map(_body, mesh=self.mesh, in_specs=in_specs,
                      out_specs=out_specs, check_rep=False),
            donate_argnums=donate, keep_unused=True)
        self.sharding = NamedSharding(self.mesh, PartitionSpec("core"))

    def stage_inputs(self, in_maps):
        jax = self.jax
        if self.dbg_name is not None:
            in_maps = [{**m, self.dbg_name: np.zeros((1, 2), np.uint32)}
                       for m in in_maps]
        staged = []
        for i in range(self.n_params):
            name = self.in_names[i]
            arr = np.concatenate([np.asarray(m[name]) for m in in_maps], axis=0)
            staged.append(jax.device_put(arr, self.sharding))
        jax.block_until_ready(staged)
        self.staged = staged

    def _zeros(self):
        jax = self.jax
        zs = [jax.device_put(
            np.zeros((self.n_cores * z.shape[0], *z.shape[1:]), z.dtype),
            self.sharding) for z in self.zero_outs]
        jax.block_until_ready(zs)
        return zs

    def run(self):
        jax = self.jax
        outs = self.sharded(*self.staged, *self._zeros())
        jax.block_until_ready(outs)
        return [
            {name: np.asarray(outs[i]).reshape(self.n_cores, *self.out_avals[i].shape)[k]
             for i, name in enumerate(self.out_names)}
            for k in range(self.n_cores)
        ]

    def bench(self, iters=20, warmup=3):
        import time
        jax = self.jax
        times = []
        for it in range(warmup + iters):
            zs = self._zeros()
            t0 = time.perf_counter()
            outs = self.sharded(*self.staged, *zs)
            jax.block_until_ready(outs)
            dt = time.perf_counter() - t0
            if it >= warmup:
                times.append(dt)
            del outs
        times = np.array(times)
        return {"min_s": float(times.min()), "med_s": float(np.median(times)),
                "mean_s": float(times.mean()), "n": iters}


_CACHE = {}


def kernel(**inputs) -> np.ndarray:
    cfg = Cfg()
    in_maps = prepare(inputs, cfg)
    key = (cfg.T, cfg.tiles_bc.tobytes())
    if key not in _CACHE:
        nc = build_program(cfg)
        _CACHE[key] = (nc, SpmdRunner(nc, cfg.n_cores))
    nc, runner = _CACHE[key]
    runner.stage_inputs(in_maps)
    res = runner.run()
    out = np.empty((cfg.B, 2), np.float32)
    for k in range(cfg.n_cores):
        out[k * cfg.B_loc:(k + 1) * cfg.B_loc] = res[k]["out"]
    return out


# revision 65
# speedup vs baseline: 2.4696x; 2.4696x over previous
"""Trainium2 Bass kernel for MAGNN link prediction (nn_MAGNN_lp).

Sharding: the B=8192 targets are split across 8 cores (1024 each) and each
core owns the metapath instances whose target falls in its range, so the
segment softmax/sum is core-local. Node towers are sharded by node rows
(5000/core, padded to 5120); the projected node table is AllGathered in
fp16 and stored as PAIRED rows [20480, 128] so every dma_gather element is
the 256B hardware minimum with no lo/hi split (idx = row>>1 fits int16;
instances are class-sorted by the parity triple of their 3 node rows so the
64-column slice offset of each tile run is compile-time).

Per chunk each core gathers all 3 metapath positions in ONE call, computes
(on fp16 DVE ops) u = ed0+ed2, the attention logit via the rotation pushed
into the attention vector (e = u.a + ed1.rot_adj(a)), exp(e-6) (global exp
shift; cancels in the softmax), and dma_scatter_adds the 136-float payload
[w*u | w*ed1 | w] into a per-target DRAM accumulator (trailing trash row
absorbs padding instances). The per-target rotation of sum(w*ed1) is applied
once per target in the head (rotation is linear), followed by normalize,
ELU, semantic attention (one tiny AllReduce), the product MLP and softmax.
Host work is integer packing of index tensors only.
"""
import numpy as np

import concourse.bass as bass
import concourse.mybir as mybir
import concourse.tile as tile
from concourse import bacc
from concourse.masks import make_identity
from dataclasses import dataclass

F32 = mybir.dt.float32
F16 = mybir.dt.float16
I16 = mybir.dt.int16
I32 = mybir.dt.int32
AF = mybir.ActivationFunctionType
ALU = mybir.AluOpType
PSUM = "PSUM"

ESHIFT = 6.0           # exp(e - ESHIFT); cancels in softmax ratio
EPS_S = 1e-9 * float(np.exp(-ESHIFT))


@dataclass
class Cfg:
    n_cores: int = 8
    B: int = 8192
    HID: int = 64
    H: int = 8
    D: int = 8
    F0: int = 512
    AV: int = 128
    CH: int = 128
    nodes_real: int = 5000      # real nodes per core
    nodes_core: int = 5120      # padded to 128
    Tc: int = 24                # max tiles per gather/scatter call
    S: int = 8                  # accumulator slots per target (HW scatter-add
                                # loses colliding updates; slots make every
                                # row within one call unique)
    T: int = 200                # tiles per metapath per core (plan sets)
    n_mp: int = 4
    gelu: bool = True           # False: Tanh stand-in (CoreSim lacks Gelu)
    dbg: bool = False
    tiles_bc: np.ndarray | None = None   # [n_bands, 8] maxed over mp, core

    @property
    def B_loc(self):
        return self.B // self.n_cores

    @property
    def n_rows(self):
        return self.nodes_core * self.n_cores    # 40960

    @property
    def n_pairs(self):
        return self.n_rows // 2                  # 20480

    @property
    def node_tiles(self):
        return self.nodes_core // 128            # 40

    @property
    def E_loc(self):
        return self.T * 128

    @property
    def kF(self):
        return self.F0 // 128

    @property
    def b_tiles(self):
        return self.B_loc // 128

    @property
    def trash_row(self):
        return self.B_loc * self.S

    @property
    def acc_rows(self):
        return self.B_loc * self.S + 128         # trailing trash rows

    @property
    def acc_step(self):
        return 192                               # 768B row stride (256B mult)

    @property
    def calls(self):
        """[(band, tile_off_in_band, ntiles)] — call windows, band-aligned."""
        out = []
        for b in range(self.tiles_bc.shape[0]):
            nb = int(self.tiles_bc[b].sum())
            off = 0
            while off < nb:
                take = min(self.Tc, nb - off)
                out.append((b, off, take))
                off += take
        return out


PAYW = 136     # payload floats per instance: w*u(64) | w*ed1(64) | w(8)

# flat layout of the packed small-weights blob (all f32)
_WB_FIELDS = [
    ("pw", 512 * 64), ("pb", 64), ("w2", 64 * 64), ("b2", 64), ("g", 64),
    ("be", 64), ("rvec", 64), ("attn", 4 * 64),
    ("suw1", 64 * 128), ("sub1", 128), ("suw2", 128),
    ("siw1", 64 * 128), ("sib1", 128), ("siw2", 128),
    ("cw1", 64 * 128), ("cb1", 128), ("cw2t", 2 * 128),
]
WB_OFF = {}
_o = 0
for _nm, _sz in _WB_FIELDS:
    WB_OFF[_nm] = _o
    _o += _sz
WB_TOTAL = _o


def _call_runs(tiles_bc, calls):
    """runs per call: [(toff_in_call, ntiles, cls)] from band class layout."""
    out = []
    for (b, off, nt) in calls:
        runs, t = [], 0
        for cls in range(8):
            n = int(tiles_bc[b][cls])
            a0, a1 = max(off, t), min(off + nt, t + n)
            if a0 < a1:
                runs.append((a0 - off, a1 - a0, cls))
            t += n
        out.append(runs)
    return out


def _ap_with(ap, offset_delta, tail_dims):
    """Copy an AP, keeping its partition dim, replacing trailing free dims."""
    return bass.AP(ap.tensor, ap.offset + offset_delta,
                   [list(ap.ap[0])] + [list(d) for d in tail_dims])


def build_program(cfg: Cfg):
    c = cfg
    assert c.tiles_bc is not None
    nc = bacc.Bacc("TRN2", target_bir_lowering=False, debug=False,
                   num_devices=c.n_cores, num_swdge_queues=2)

    def di(name, shape, dtype=F32):
        return nc.dram_tensor(name, list(shape), dtype, kind="ExternalInput")

    T8 = c.T * 8
    feats = di("feats", (c.nodes_core, c.F0))
    wblob = di("wblob", (WB_TOTAL,))
    emi16 = di("emi16", (c.n_mp * 128, 3 * T8), I16)
    tl16 = di("tl16", (c.n_mp * 128, T8), I16)

    def wb(nm, dims, extra=0):
        return bass.AP(wblob.ap().tensor, WB_OFF[nm] + extra,
                       [list(d) for d in dims])

    def wbrow(nm, n, extra=0):
        return wb(nm, [[n, 1], [1, n]], extra)

    outd = nc.dram_tensor("out", [c.B_loc, 2], F32, kind="ExternalOutput")
    if c.dbg:
        dbg_acc = nc.dram_tensor("dbg_acc", [c.B_loc, PAYW], F32,
                                 kind="ExternalOutput")
        dbg_tab = nc.dram_tensor("dbg_tab", [2048, 64], F16,
                                 kind="ExternalOutput")
        dbg_ed = nc.dram_tensor("dbg_ed", [128, 384], F16,
                                kind="ExternalOutput")

    HID, H, D, Tc = c.HID, c.H, c.D, c.Tc

    with tile.TileContext(nc) as tc:
        with (
            tc.tile_pool(name="const", bufs=1) as kpool,
            tc.tile_pool(name="dram", bufs=1, space="DRAM") as dpool,
        ):
            pk_ctx = tc.tile_pool(name="ps_const", bufs=1, space="PSUM")
            pkpool = pk_ctx.__enter__()
            # ---------- constants ----------
            id128 = kpool.tile([128, 128], F32, tag="id128")
            make_identity(nc, id128[:])
            ones1 = kpool.tile([1, 128], F32, tag="ones1")
            nc.vector.memset(ones1[:], 1.0)
            onescol = kpool.tile([128, 1], F32, tag="onescol")
            nc.vector.memset(onescol[:], 1.0)
            epscol = kpool.tile([128, 1], F32, tag="epscol")
            nc.vector.memset(epscol[:], 1e-5)
            shiftcol = kpool.tile([128, 1], F32, tag="shiftcol")
            nc.vector.memset(shiftcol[:], -ESHIFT)
            zacc = kpool.tile([128, 1560], F32, tag="zacc")
            nc.vector.memset(zacc[:], 0.0)

            def rep_row(dram_vec, n, scale=None, tag=None, dtype=F32):
                row = kpool.tile([1, n], F32, tag=f"{tag}_row")
                nc.sync.dma_start(row[:], dram_vec)
                return rep_from_row(row[:], n, tag, scale=scale, dtype=dtype)

            def rep_from_row(row_ap, n, tag, scale=None, dtype=F32):
                ps = pkpool.tile([128, 512], F32, space=PSUM, tag="reppsum")
                nc.tensor.matmul(out=ps[:, :n], lhsT=ones1[:], rhs=row_ap,
                                 start=True, stop=True)
                rep = kpool.tile([128, n], dtype, tag=tag)
                if scale is None:
                    nc.vector.tensor_copy(rep[:], ps[:, :n])
                else:
                    nc.vector.tensor_scalar_mul(rep[:], ps[:, :n], scale)
                return rep

            PBrep = rep_row(wbrow("pb", HID), HID, tag="PBrep")
            B2rep = rep_row(wbrow("b2", HID), HID, tag="B2rep")
            G3rep = rep_row(wbrow("g", HID), HID, scale=1.0 / 3.0, tag="G3rep")
            BE3rep = rep_row(wbrow("be", HID), HID, scale=1.0 / 3.0, tag="BE3rep")
            SUB1rep = rep_row(wbrow("sub1", c.AV), c.AV, tag="SUB1rep")
            SIB1rep = rep_row(wbrow("sib1", c.AV), c.AV, tag="SIB1rep")
            SUW2rep = rep_row(wbrow("suw2", c.AV), c.AV, tag="SUW2rep")
            SIW2rep = rep_row(wbrow("siw2", c.AV), c.AV, tag="SIW2rep")
            CB1rep = rep_row(wbrow("cb1", c.CH), c.CH, tag="CB1rep")
            CW20rep = rep_row(wbrow("cw2t", c.CH), c.CH, tag="CW20rep")
            CW21rep = rep_row(wbrow("cw2t", c.CH, extra=c.CH), c.CH, tag="CW21rep")

            # ---------- rotation constants (normalize r on device) ----------
            # (same construction as before: crrow = Re(r) per feature,
            # c2urow/c2irow = +/- Im(r) with per-parity sign)
            rcol = kpool.tile([HID, 1], F32, tag="rcol")
            nc.sync.dma_start(rcol[:], wb("rvec", [[1, HID], [1, 1]]))
            idh = kpool.tile([HID, HID], F32, tag="idh")
            make_identity(nc, idh[:])
            Sp = kpool.tile([HID, HID], F32, tag="Sp")
            nc.vector.memset(Sp[:], 0.0)
            nc.vector.tensor_copy(Sp[:, 1:HID], idh[:, 0:HID - 1])
            Sm = kpool.tile([HID, HID], F32, tag="Sm")
            nc.vector.memset(Sm[:], 0.0)
            nc.vector.tensor_copy(Sm[:, 0:HID - 1], idh[:, 1:HID])
            pidx = kpool.tile([HID, 1], I32, tag="pidx")
            nc.gpsimd.iota(pidx[:], pattern=[[0, 1]], base=0, channel_multiplier=1)
            podd_i = kpool.tile([HID, 1], I32, tag="podd_i")
            nc.vector.tensor_scalar(podd_i[:], pidx[:], 1, None, ALU.bitwise_and)
            podd = kpool.tile([HID, 1], F32, tag="podd")
            nc.vector.tensor_copy(podd[:], podd_i[:])
            peven = kpool.tile([HID, 1], F32, tag="peven")
            nc.vector.tensor_scalar(peven[:], podd[:], -1.0, -1.0, ALU.add, ALU.mult)
            Spe = kpool.tile([HID, HID], F32, tag="Spe")
            nc.vector.tensor_scalar_mul(Spe[:], Sp[:], peven[:])
            Smo = kpool.tile([HID, HID], F32, tag="Smo")
            nc.vector.tensor_scalar_mul(Smo[:], Sm[:], podd[:])
            Ie = kpool.tile([HID, HID], F32, tag="Ie")
            nc.vector.tensor_scalar_mul(Ie[:], idh[:], peven[:])
            Io = kpool.tile([HID, HID], F32, tag="Io")
            nc.vector.tensor_scalar_mul(Io[:], idh[:], podd[:])
            M2 = kpool.tile([HID, HID], F32, tag="M2")
            nc.vector.tensor_tensor(M2[:], idh[:], Spe[:], ALU.add)
            nc.vector.tensor_tensor(M2[:], M2[:], Smo[:], ALU.add)
            Me = kpool.tile([HID, HID], F32, tag="Me")
            nc.vector.tensor_tensor(Me[:], Ie[:], Spe[:], ALU.add)
            Mo = kpool.tile([HID, HID], F32, tag="Mo")
            nc.vector.tensor_tensor(Mo[:], Io[:], Smo[:], ALU.add)
            sqc = kpool.tile([HID, 1], F32, tag="sqc")
            nc.vector.tensor_tensor(sqc[:], rcol[:], rcol[:], ALU.mult)
            n2 = pkpool.tile([HID, 1], F32, space=PSUM, tag="n2")
            nc.tensor.matmul(out=n2[:], lhsT=M2[:], rhs=sqc[:], start=True, stop=True)
            nrm = kpool.tile([HID, 1], F32, tag="nrm")
            nc.scalar.activation(nrm[:], n2[:], AF.Sqrt)
            invn = kpool.tile([HID, 1], F32, tag="invn")
            nc.vector.reciprocal(invn[:], nrm[:])
            rn = kpool.tile([HID, 1], F32, tag="rn")
            nc.vector.tensor_scalar_mul(rn[:], rcol[:], invn[:])
            cr2 = pkpool.tile([HID, 1], F32, space=PSUM, tag="cr2")
            nc.tensor.matmul(out=cr2[:], lhsT=Me[:], rhs=rn[:], start=True, stop=True)
            ci2 = pkpool.tile([HID, 1], F32, space=PSUM, tag="ci2")
            nc.tensor.matmul(out=ci2[:], lhsT=Mo[:], rhs=rn[:], start=True, stop=True)
            cr2s = kpool.tile([HID, 1], F32, tag="cr2s")
            nc.vector.tensor_copy(cr2s[:], cr2[:])
            ci2s = kpool.tile([HID, 1], F32, tag="ci2s")
            nc.vector.tensor_copy(ci2s[:], ci2[:])
            crrow_ps = pkpool.tile([1, HID], F32, space=PSUM, tag="crrow_ps")
            nc.tensor.matmul(out=crrow_ps[:], lhsT=cr2s[:], rhs=idh[:], start=True, stop=True)
            crrow = kpool.tile([1, HID], F32, tag="crrow")
            nc.vector.tensor_copy(crrow[:], crrow_ps[:])
            cirow_ps = pkpool.tile([1, HID], F32, space=PSUM, tag="cirow_ps")
            nc.tensor.matmul(out=cirow_ps[:], lhsT=ci2s[:], rhs=idh[:], start=True, stop=True)
            cirow = kpool.tile([1, HID], F32, tag="cirow")
            nc.vector.tensor_copy(cirow[:], cirow_ps[:])
            fidx = kpool.tile([1, HID], I32, tag="fidx")
            nc.gpsimd.iota(fidx[:], pattern=[[1, HID]], base=0, channel_multiplier=0)
            fodd_i = kpool.tile([1, HID], I32, tag="fodd_i")
            nc.vector.tensor_scalar(fodd_i[:], fidx[:], 1, None, ALU.bitwise_and)
            fsign = kpool.tile([1, HID], F32, tag="fsign")
            nc.vector.tensor_copy(fsign[:], fodd_i[:])
            nc.vector.tensor_scalar(fsign[:], fsign[:], -2.0, 1.0, ALU.mult, ALU.add)
            c2u = kpool.tile([1, HID], F32, tag="c2u")
            c2i = kpool.tile([1, HID], F32, tag="c2i")
            c2row = [c2u, c2i]
            nc.vector.tensor_tensor(c2row[0][:], cirow[:], fsign[:], ALU.mult)
            nc.vector.tensor_scalar_mul(c2row[1][:], c2row[0][:], -1.0)
            C1rep = rep_from_row(crrow[:], HID, "C1rep")
            C2rep = [rep_from_row(c2row[0][:], HID, "C2urep"),
                     rep_from_row(c2row[1][:], HID, "C2irep")]

            # attention rows (fp16 replicated) + rotation-adjoint rows:
            # a2_d = C1_d * a_d + C2_{d^1} * a_{d^1}
            ATT16, A2_16 = [], []
            for mp in range(c.n_mp):
                side = 0 if mp < 2 else 1
                arow = kpool.tile([1, HID], F32, tag=f"arow{mp}")
                nc.sync.dma_start(arow[:], wbrow("attn", HID, extra=mp * HID))
                m = kpool.tile([1, HID], F32, tag=f"m{mp}")
                nc.vector.tensor_tensor(m[:], c2row[side][:], arow[:], ALU.mult)
                a2 = kpool.tile([1, HID], F32, tag=f"a2_{mp}")
                nc.vector.tensor_tensor(a2[:], crrow[:], arow[:], ALU.mult)
                mswap = _ap_with(m[:], 1, [[2, HID // 2], [-1, 2]])
                nc.vector.tensor_tensor(a2[:], a2[:], mswap, ALU.add)
                ATT16.append(rep_from_row(arow[:], HID, f"AT16_{mp}", dtype=F16))
                A2_16.append(rep_from_row(a2[:], HID, f"A216_{mp}", dtype=F16))

            pwsb = kpool.tile([128, c.kF, HID], F32, tag="pwsb")
            nc.sync.dma_start(
                pwsb[:], wb("pw", [[HID, 128], [128 * HID, c.kF], [1, HID]]))
            w2sb = kpool.tile([HID, HID], F32, tag="w2sb")
            nc.sync.dma_start(w2sb[:], wb("w2", [[HID, HID], [1, HID]]))
            suw1sb = kpool.tile([HID, c.AV], F32, tag="suw1sb")
            nc.sync.dma_start(suw1sb[:], wb("suw1", [[c.AV, HID], [1, c.AV]]))
            siw1sb = kpool.tile([HID, c.AV], F32, tag="siw1sb")
            nc.sync.dma_start(siw1sb[:], wb("siw1", [[c.AV, HID], [1, c.AV]]))
            cw1sb = kpool.tile([HID, c.CH], F32, tag="cw1sb")
            nc.sync.dma_start(cw1sb[:], wb("cw1", [[c.CH, HID], [1, c.CH]]))

            pk_ctx.__exit__(None, None, None)

            # ---------- dram tiles ----------
            tower_t = dpool.tile([c.nodes_core, HID], F16, tag="tower")
            table_t = dpool.tile([c.n_rows, HID], F16, tag="table")
            accs = [dpool.tile([c.acc_rows, c.acc_step], F32, tag=f"acc{mp}",
                               name=f"acc{mp}")
                    for mp in range(c.n_mp)]
            # acc_rows*acc_step == 128*12480; zero in 8 contiguous strips
            assert c.acc_rows * c.acc_step == 128 * 12480
            for mp in range(c.n_mp):
                for j in range(8):
                    dst = bass.AP(accs[mp][:].tensor,
                                  accs[mp][:].offset + j * 1560,
                                  [[12480, 128], [1, 1560]])
                    nc.sync.dma_start(dst, zacc[:])

            # ---------- tower (two passes to avoid act-table thrash) ----------
            nt = c.node_tiles
            with (
                tc.tile_pool(name="tw_x", bufs=2) as xpool,
                tc.tile_pool(name="tw_ps", bufs=2, space="PSUM") as tpspool,
                tc.tile_pool(name="tw_s", bufs=2) as tspool,
                tc.tile_pool(name="tw_keep", bufs=1) as tkpool,
            ):
                Z = tkpool.tile([128, nt, HID], F32, tag="Z")
                HH = tkpool.tile([128, nt, HID], F32, tag="HH")
                YC = tkpool.tile([128, nt, HID], F32, tag="YC")
                VV = tkpool.tile([128, nt], F32, tag="VV")
                for j in range(nt):
                    xt = xpool.tile([128, c.F0], F32, tag="xt")
                    nc.sync.dma_start(xt[:], feats.ap()[j * 128:(j + 1) * 128, :])
                    xT = xpool.tile([128, c.kF, 128], F32, tag="xT")
                    for kk in range(c.kF):
                        pst = tpspool.tile([128, 128], F32, space=PSUM, tag="pst")
                        nc.tensor.transpose(pst[:], xt[:, kk * 128:(kk + 1) * 128], id128[:])
                        nc.scalar.activation(xT[:, kk, :], pst[:], AF.Copy)
                    z = tpspool.tile([128, HID], F32, space=PSUM, tag="z")
                    for kk in range(c.kF):
                        nc.tensor.matmul(out=z[:], lhsT=xT[:, kk, :], rhs=pwsb[:, kk, :],
                                         start=(kk == 0), stop=(kk == c.kF - 1))
                    nc.vector.tensor_tensor(Z[:, j, :], z[:], PBrep[:], ALU.add)
                # one activation call for the whole tower
                nc.scalar.activation(HH[:].rearrange("p a b -> p (a b)"),
                                     Z[:].rearrange("p a b -> p (a b)"),
                                     AF.Gelu if c.gelu else AF.Tanh)
                for j in range(nt):
                    hT_ps = tpspool.tile([HID, 128], F32, space=PSUM, tag="hT_ps")
                    nc.tensor.transpose(hT_ps[:], HH[:, j, :], id128[:])
                    hT = tspool.tile([HID, 128], F32, tag="hT")
                    nc.scalar.activation(hT[:], hT_ps[:], AF.Copy)
                    y = tpspool.tile([128, HID], F32, space=PSUM, tag="y")
                    nc.tensor.matmul(out=y[:], lhsT=hT[:], rhs=w2sb[:], start=True, stop=True)
                    ys = tspool.tile([128, HID], F32, tag="ys")
                    nc.vector.tensor_tensor(ys[:], y[:], B2rep[:], ALU.add)
                    nc.vector.tensor_tensor(ys[:], ys[:], Z[:, j, :], ALU.add)
                    mu = tspool.tile([128, 1], F32, tag="mu")
                    nc.vector.tensor_reduce(mu[:], ys[:], mybir.AxisListType.X, ALU.add)
                    nc.vector.tensor_scalar_mul(mu[:], mu[:], 1.0 / HID)
                    nc.vector.tensor_scalar(YC[:, j, :], ys[:], mu[:], None, ALU.subtract)
                    sq = tspool.tile([128, HID], F32, tag="sq")
                    nc.vector.tensor_tensor(sq[:], YC[:, j, :], YC[:, j, :], ALU.mult)
                    nc.vector.tensor_reduce(VV[:, j:j + 1], sq[:], mybir.AxisListType.X, ALU.add)
                sdv = tspool.tile([128, nt], F32, tag="sdv")
                nc.scalar.activation(sdv[:], VV[:], AF.Sqrt, bias=epscol[:],
                                     scale=1.0 / HID)
                inv = tspool.tile([128, nt], F32, tag="inv")
                nc.vector.reciprocal(inv[:], sdv[:])
                t1 = tkpool.tile([128, nt, HID], F32, tag="t1")
                invb = _ap_with(inv[:], 0, [[1, nt], [0, HID]])
                nc.vector.tensor_tensor(t1[:], YC[:], invb, ALU.mult)
                g3b = _ap_with(G3rep[:], 0, [[0, nt], [1, HID]])
                be3b = _ap_with(BE3rep[:], 0, [[0, nt], [1, HID]])
                TS = tkpool.tile([128, nt, HID], F16, tag="TS")
                nc.vector.tensor_tensor(TS[:], t1[:], g3b, ALU.mult)
                nc.vector.tensor_tensor(TS[:], TS[:], be3b, ALU.add)
                # SBUF [128, nt, HID] -> DRAM rows (j*128+p)
                dst = bass.AP(tower_t[:].tensor, tower_t[:].offset,
                              [[HID, 128], [128 * HID, nt], [1, HID]])
                nc.sync.dma_start(dst, TS[:])

            nc.gpsimd.collective_compute(
                "AllGather", ALU.bypass,
                replica_groups=[list(range(c.n_cores))],
                ins=[tower_t.opt()], outs=[table_t.opt()],
            )
            # paired-row view for gathers: [n_pairs, 128] fp16
            table_pairs = bass.AP(table_t[:].tensor, table_t[:].offset,
                                  [[128, c.n_pairs], [1, 128]])

            # ---------- metapath chunks: gather, logits, scatter-add ----------
            calls = c.calls
            runs_all = _call_runs(c.tiles_bc, calls)
            # per-call column offsets into the packed emi16 / tl16 streams
            emi_off, tl_off = [0], [0]
            for (_, _, nt) in calls:
                emi_off.append(emi_off[-1] + 3 * nt * 8)
                tl_off.append(tl_off[-1] + nt * 8)
            with (
                tc.tile_pool(name="mp_idx", bufs=2) as ipool,
                tc.tile_pool(name="mp_ed", bufs=3) as edpool,
                tc.tile_pool(name="mp_row", bufs=2) as rowpool,
                tc.tile_pool(name="mp_tmp", bufs=1) as mtpool,
                tc.tile_pool(name="mp_tl", bufs=1) as tlpool,
                tc.tile_pool(name="hd_acc", bufs=2) as apool,
                tc.tile_pool(name="hd_b", bufs=1) as bhpool,
                tc.tile_pool(name="hd_s", bufs=3) as hpool,
                tc.tile_pool(name="hd_ps", bufs=1, space="PSUM") as hpspool,
                tc.tile_pool(name="hd_keep", bufs=1) as keep,
            ):
                outs_all = keep.tile([128, c.n_mp, c.b_tiles, HID], F32, tag="outs_all")
                acc4 = keep.tile([1, c.n_mp], F32, tag="acc4")
                nc.vector.memset(acc4[:], 0.0)

                emi_sbs, tl_sbs, eds = {}, {}, {}
                gsem = nc.alloc_semaphore("gsem")
                ssem = nc.alloc_semaphore("ssem")

                def emit_gather(mp, ci, direct=True):
                    _, _, nt = calls[ci]
                    ed = edpool.tile([128, 3 * Tc, 128], F16, tag="ed")
                    eds[(mp, ci)] = ed
                    nc.gpsimd.dma_gather(
                        out_ap=ed[:, 0:3 * nt, :], in_ap=table_pairs,
                        idxs_ap=emi_sbs[mp][:, emi_off[ci]:emi_off[ci + 1]],
                        num_idxs=3 * nt * 128, num_idxs_reg=3 * nt * 128,
                        elem_size=128, single_packet=False)

                def emit_compute_scatter(mp, ci):
                    ed = eds.pop((mp, ci))
                    _, _, nt = calls[ci]
                    runs = runs_all[ci]
                    u = mtpool.tile([128, Tc, HID], F16, tag="u")
                    v = mtpool.tile([128, Tc, HID], F16, tag="v")
                    t2 = mtpool.tile([128, Tc, HID], F16, tag="t2")
                    for (t0, ntk, cls) in runs:
                        p0, p1, p2 = cls & 1, (cls >> 1) & 1, (cls >> 2) & 1
                        e0 = ed[:, t0:t0 + ntk, p0 * HID:p0 * HID + HID]
                        e1 = ed[:, nt + t0:nt + t0 + ntk, p1 * HID:p1 * HID + HID]
                        e2 = ed[:, 2 * nt + t0:2 * nt + t0 + ntk, p2 * HID:p2 * HID + HID]
                        nc.vector.tensor_tensor(u[:, t0:t0 + ntk, :], e0, e2, ALU.add)
                        a2b = _ap_with(A2_16[mp][:], 0, [[0, ntk], [1, HID]])
                        nc.vector.tensor_tensor(t2[:, t0:t0 + ntk, :], e1, a2b, ALU.mult)
                    ab = _ap_with(ATT16[mp][:], 0, [[0, nt], [1, HID]])
                    nc.vector.tensor_tensor(v[:, 0:nt, :], u[:, 0:nt, :], ab, ALU.mult)
                    nc.vector.tensor_tensor(v[:, 0:nt, :], v[:, 0:nt, :],
                                            t2[:, 0:nt, :], ALU.add)
                    e32 = mtpool.tile([128, Tc, H], F32, tag="e32")
                    nc.vector.tensor_reduce(
                        e32[:, 0:nt, :],
                        v[:, 0:nt, :].rearrange("p t (h d) -> p t h d", d=D),
                        mybir.AxisListType.X, ALU.add)
                    el = mtpool.tile([128, Tc, H], F32, tag="el")
                    nc.vector.tensor_scalar_mul(el[:, 0:nt, :], e32[:, 0:nt, :], 0.01)
                    nc.vector.tensor_tensor(el[:, 0:nt, :], el[:, 0:nt, :],
                                            e32[:, 0:nt, :], ALU.max)
                    w16 = mtpool.tile([128, Tc, H], F16, tag="w16")
                    nc.scalar.activation(w16[:, 0:nt, :], el[:, 0:nt, :], AF.Exp,
                                         bias=shiftcol[:])
                    rows = rowpool.tile([128, Tc, PAYW], F32, tag="rows")
                    wb = _ap_with(w16[:], 0, [[H, nt], [1, H], [0, D]])
                    nc.vector.tensor_tensor(rows[:, 0:nt, 0:HID], u[:, 0:nt, :],
                                            wb, ALU.mult)
                    nc.vector.tensor_copy(rows[:, 0:nt, 2 * HID:PAYW], w16[:, 0:nt, :])
                    for (t0, ntk, cls) in runs:
                        p1 = (cls >> 1) & 1
                        e1 = ed[:, nt + t0:nt + t0 + ntk, p1 * HID:p1 * HID + HID]
                        wbr = _ap_with(w16[:], t0 * H, [[H, ntk], [1, H], [0, D]])
                        nc.vector.tensor_tensor(rows[:, t0:t0 + ntk, HID:2 * HID],
                                                e1, wbr, ALU.mult)
                    acc_ap = bass.AP(accs[mp][:].tensor, accs[mp][:].offset,
                                     [[c.acc_step, c.acc_rows], [1, PAYW]])
                    nc.gpsimd.dma_scatter_add(
                        out_ap=acc_ap, in_ap=rows[:, 0:nt, :],
                        idxs_ap=tl_sbs[mp][:, tl_off[ci]:tl_off[ci + 1]],
                        num_idxs=nt * 128, num_idxs_reg=nt * 128,
                        elem_size=PAYW, elem_step=c.acc_step,
                        single_packet=False)

                def emit_head(mp):
                    side = 0 if mp < 2 else 1
                    w1sb = suw1sb if mp < 2 else siw1sb
                    b1rep = SUB1rep if mp < 2 else SIB1rep
                    w2rep = SUW2rep if mp < 2 else SIW2rep
                    bts = c.b_tiles
                    f1 = apool.tile([128, bts, PAYW], F32, tag="f1")
                    for bt in range(bts):
                        acc_sb = apool.tile([128, c.S, PAYW], F32, tag="acc_sb")
                        src = bass.AP(accs[mp][:].tensor,
                                      accs[mp][:].offset
                                      + bt * 128 * c.S * c.acc_step,
                                      [[c.S * c.acc_step, 128],
                                       [c.acc_step, c.S], [1, PAYW]])
                        nc.sync.dma_start(acc_sb[:], src)
                        f4 = apool.tile([128, 4, PAYW], F32, tag="f4")
                        nc.vector.tensor_tensor(f4[:], acc_sb[:, 0:4, :],
                                                acc_sb[:, 4:8, :], ALU.add)
                        nc.vector.tensor_tensor(f4[:, 0:2, :], f4[:, 0:2, :],
                                                f4[:, 2:4, :], ALU.add)
                        nc.vector.tensor_tensor(f1[:, bt, :], f4[:, 0, :],
                                                f4[:, 1, :], ALU.add)
                    # batched over all b_tiles: [128, bts, *]
                    s1a = _ap_with(f1[:], HID, [[PAYW, bts], [1, HID]])
                    s02a = _ap_with(f1[:], 0, [[PAYW, bts], [1, HID]])
                    swa = _ap_with(f1[:], 2 * HID, [[PAYW, bts], [1, H]])
                    den = bhpool.tile([128, bts, H], F32, tag="den")
                    nc.vector.tensor_scalar_add(den[:], swa, EPS_S)
                    dinv = bhpool.tile([128, bts, H], F32, tag="dinv")
                    nc.vector.reciprocal(dinv[:], den[:])
                    rot = bhpool.tile([128, bts, HID], F32, tag="rot")
                    c1b = _ap_with(C1rep[:], 0, [[0, bts], [1, HID]])
                    nc.vector.tensor_tensor(rot[:], s1a, c1b, ALU.mult)
                    tb = bhpool.tile([128, bts, HID], F32, tag="tb")
                    s1swap = _ap_with(f1[:], HID + 1,
                                      [[PAYW, bts], [2, HID // 2], [-1, 2]])
                    c2b = _ap_with(C2rep[side][:], 0, [[0, bts], [1, HID]])
                    nc.vector.tensor_tensor(tb[:], s1swap, c2b, ALU.mult)
                    nc.vector.tensor_tensor(rot[:], rot[:], tb[:], ALU.add)
                    nc.vector.tensor_tensor(rot[:], rot[:], s02a, ALU.add)
                    ret = bhpool.tile([128, bts, HID], F32, tag="ret")
                    dinvb = _ap_with(dinv[:], 0, [[H, bts], [1, H], [0, D]])
                    nc.vector.tensor_tensor(ret[:], rot[:], dinvb, ALU.mult)
                    neg = bhpool.tile([128, bts, HID], F32, tag="neg")
                    nc.vector.tensor_scalar_min(neg[:], ret[:], 0.0)
                    en = bhpool.tile([128, bts, HID], F32, tag="en")
                    nc.scalar.activation(en[:], neg[:], AF.Exp)
                    o_all = outs_all[:, mp, :, :]
                    nc.vector.tensor_scalar_max(ret[:], ret[:], 0.0)
                    nc.vector.tensor_scalar_add(en[:], en[:], -1.0)
                    nc.vector.tensor_tensor(o_all, ret[:], en[:], ALU.add)
                    for bt in range(bts):
                        o = outs_all[:, mp, bt, :]
                        oT_ps = hpspool.tile([HID, 128], F32, space=PSUM, tag="oT_ps")
                        nc.tensor.transpose(oT_ps[:], o, id128[:])
                        oT = hpool.tile([HID, 128], F32, tag="oT")
                        nc.scalar.activation(oT[:], oT_ps[:], AF.Copy)
                        tt = hpspool.tile([128, c.AV], F32, space=PSUM, tag="tt")
                        nc.tensor.matmul(out=tt[:], lhsT=oT[:], rhs=w1sb[:], start=True, stop=True)
                        th = hpool.tile([128, c.AV], F32, tag="th")
                        nc.vector.tensor_tensor(th[:], tt[:], b1rep[:], ALU.add)
                        nc.scalar.activation(th[:], th[:], AF.Tanh)
                        nc.vector.tensor_tensor(th[:], th[:], w2rep[:], ALU.mult)
                        rsum = hpool.tile([128, 1], F32, tag="rsum")
                        nc.vector.tensor_reduce(rsum[:], th[:], mybir.AxisListType.X, ALU.add)
                        sp = hpspool.tile([1, 1], F32, space=PSUM, tag="sp")
                        nc.tensor.matmul(out=sp[:], lhsT=rsum[:], rhs=onescol[:], start=True, stop=True)
                        nc.vector.tensor_tensor(acc4[:, mp:mp + 1], acc4[:, mp:mp + 1], sp[:], ALU.add)

                # software-pipelined emission: gather(ci+1) before compute(ci)
                for mp in range(c.n_mp):
                    emi_sb = ipool.tile([128, 3 * T8], I16, tag="emi_sb")
                    nc.sync.dma_start(
                        emi_sb[:], emi16.ap()[mp * 128:(mp + 1) * 128, :])
                    emi_sbs[mp] = emi_sb
                    tl_sb = tlpool.tile([128, T8], I16, tag="tl_sb")
                    nc.sync.dma_start(tl_sb[:], tl16.ap()[mp * 128:(mp + 1) * 128, :])
                    tl_sbs[mp] = tl_sb
                    for ci in range(len(calls)):
                        emit_gather(mp, ci)
                        if ci > 1:
                            emit_compute_scatter(mp, ci - 2)
                    emit_compute_scatter(mp, len(calls) - 2)
                    emit_compute_scatter(mp, len(calls) - 1)
                    emit_head(mp)

                if c.dbg:
                    dbg_ctx = tc.tile_pool(name="dbgp", bufs=1)
                    dpool_dbg = dbg_ctx.__enter__()
                    for b0 in range(0, c.b_tiles, 2):
                        dsb = dpool_dbg.tile([128, 2, PAYW], F32, tag="dsb",
                                             name=f"dsb{b0}")
                        src = bass.AP(accs[0][:].tensor,
                                      accs[0][:].offset + b0 * 128 * c.acc_step,
                                      [[c.acc_step, 128], [128 * c.acc_step, 2],
                                       [1, PAYW]])
                        nc.sync.dma_start(dsb[:], src)
                        dst = bass.AP(dbg_acc.ap().tensor, b0 * 128 * PAYW,
                                      [[PAYW, 128], [128 * PAYW, 2], [1, PAYW]])
                        nc.sync.dma_start(dst, dsb[:])
                    tsb = dpool_dbg.tile([128, 16, 64], F16, tag="tsb")
                    nc.sync.dma_start(
                        tsb[:], bass.AP(table_t[:].tensor, 0,
                                        [[64, 128], [128 * 64, 16], [1, 64]]))
                    dst2 = bass.AP(dbg_tab.ap().tensor, 0,
                                   [[64, 128], [128 * 64, 16], [1, 64]])
                    nc.sync.dma_start(dst2, tsb[:])
                    esb = dpool_dbg.tile([128, 384], F16, tag="esb")
                    nc.gpsimd.dma_gather(
                        out_ap=esb[:].rearrange("p (a b) -> p a b", a=3),
                        in_ap=table_pairs,
                        idxs_ap=emi_sbs[c.n_mp - 1][:, 0:24],
                        num_idxs=384, num_idxs_reg=384,
                        elem_size=128, single_packet=False)
                    nc.sync.dma_start(dbg_ed.ap(), esb[:])
                    dbg_ctx.__exit__(None, None, None)

                # ---------- semantic softmax + product MLP ----------
                sin_t = dpool.tile([1, 128], F32, tag="sin")
                sout_t = dpool.tile([1, 128], F32, tag="sout")
                zrow = hpool.tile([1, 128], F32, tag="zrow")
                nc.vector.memset(zrow[:], 0.0)
                nc.sync.dma_start(sin_t[:], zrow[:])
                nc.sync.dma_start(sin_t[0:1, 0:c.n_mp], acc4[:])
                nc.gpsimd.collective_compute(
                    "AllReduce", ALU.add,
                    replica_groups=[list(range(c.n_cores))],
                    ins=[sin_t.opt()], outs=[sout_t.opt()],
                )
                s4 = hpool.tile([1, c.n_mp], F32, tag="s4")
                nc.sync.dma_start(s4[:], sout_t[0:1, 0:c.n_mp])
                e4 = hpool.tile([1, c.n_mp], F32, tag="e4")
                nc.scalar.activation(e4[:], s4[:], AF.Exp, scale=1.0 / c.B)
                beta = hpool.tile([1, c.n_mp], F32, tag="beta")
                for sd in range(2):
                    ssum = hpool.tile([1, 1], F32, tag="ssum")
                    nc.vector.tensor_reduce(ssum[:], e4[:, 2 * sd:2 * sd + 2],
                                            mybir.AxisListType.X, ALU.add)
                    sinv = hpool.tile([1, 1], F32, tag="sinv")
                    nc.vector.reciprocal(sinv[:], ssum[:])
                    nc.vector.tensor_scalar_mul(beta[:, 2 * sd:2 * sd + 2],
                                                e4[:, 2 * sd:2 * sd + 2], sinv[:])
                bc_ps = hpspool.tile([128, c.n_mp], F32, space=PSUM, tag="bc_ps")
                nc.tensor.matmul(out=bc_ps[:], lhsT=ones1[:], rhs=beta[:], start=True, stop=True)
                bcol = keep.tile([128, c.n_mp], F32, tag="bcol")
                nc.vector.tensor_copy(bcol[:], bc_ps[:])

                for bt in range(c.b_tiles):
                    hu = hpool.tile([128, HID], F32, tag="hu")
                    hi_ = hpool.tile([128, HID], F32, tag="hi_")
                    t0 = hpool.tile([128, HID], F32, tag="t0")
                    nc.vector.tensor_scalar_mul(hu[:], outs_all[:, 0, bt, :], bcol[:, 0:1])
                    nc.vector.tensor_scalar_mul(t0[:], outs_all[:, 1, bt, :], bcol[:, 1:2])
                    nc.vector.tensor_tensor(hu[:], hu[:], t0[:], ALU.add)
                    nc.vector.tensor_scalar_mul(hi_[:], outs_all[:, 2, bt, :], bcol[:, 2:3])
                    nc.vector.tensor_scalar_mul(t0[:], outs_all[:, 3, bt, :], bcol[:, 3:4])
                    nc.vector.tensor_tensor(hi_[:], hi_[:], t0[:], ALU.add)
                    xx = hpool.tile([128, HID], F32, tag="xx")
                    nc.vector.tensor_tensor(xx[:], hu[:], hi_[:], ALU.mult)
                    xT_ps = hpspool.tile([HID, 128], F32, space=PSUM, tag="xT_ps")
                    nc.tensor.transpose(xT_ps[:], xx[:], id128[:])
                    xT = hpool.tile([HID, 128], F32, tag="xT")
                    nc.vector.tensor_copy(xT[:], xT_ps[:])
                    yy = hpspool.tile([128, c.CH], F32, space=PSUM, tag="yy")
                    nc.tensor.matmul(out=yy[:], lhsT=xT[:], rhs=cw1sb[:], start=True, stop=True)
                    ya = hpool.tile([128, c.CH], F32, tag="ya")
                    nc.vector.tensor_tensor(ya[:], yy[:], CB1rep[:], ALU.add)
                    nc.scalar.activation(ya[:], ya[:], AF.Relu)
                    l0t = hpool.tile([128, c.CH], F32, tag="l0t")
                    nc.vector.tensor_tensor(l0t[:], ya[:], CW20rep[:], ALU.mult)
                    l0 = hpool.tile([128, 1], F32, tag="l0")
                    nc.vector.tensor_reduce(l0[:], l0t[:], mybir.AxisListType.X, ALU.add)
                    nc.vector.tensor_tensor(l0t[:], ya[:], CW21rep[:], ALU.mult)
                    l1 = hpool.tile([128, 1], F32, tag="l1")
                    nc.vector.tensor_reduce(l1[:], l0t[:], mybir.AxisListType.X, ALU.add)
                    dl = hpool.tile([128, 1], F32, tag="dl")
                    ot = hpool.tile([128, 2], F32, tag="ot")
                    nc.vector.tensor_tensor(dl[:], l0[:], l1[:], ALU.subtract)
                    nc.scalar.activation(ot[:, 0:1], dl[:], AF.Sigmoid)
                    nc.vector.tensor_tensor(dl[:], l1[:], l0[:], ALU.subtract)
                    nc.scalar.activation(ot[:, 1:2], dl[:], AF.Sigmoid)
                    nc.sync.dma_start(outd.ap()[bt * 128:(bt + 1) * 128, :], ot[:])

    nc.compile()
    return nc


# ---------------------------------------------------------------------------
# host side: sharding / packing (integer work only)
# ---------------------------------------------------------------------------

def _mp_arrays(inputs, mp):
    if mp < 2:
        return np.asarray(inputs["emi_user"][mp]), np.asarray(inputs["tgt_user"][mp])
    return np.asarray(inputs["emi_item"][mp - 2]), np.asarray(inputs["tgt_item"][mp - 2])


def _rows_of(emi, c: Cfg):
    """Global node id -> padded table row id."""
    return (emi // c.nodes_real) * c.nodes_core + emi % c.nodes_real


def _band_cls(emi, tgt, k, c: Cfg):
    """Per-core (band, cls, tloc, rows) for the selected instances."""
    lo, hi = k * c.B_loc, (k + 1) * c.B_loc
    sel = np.nonzero((tgt >= lo) & (tgt < hi))[0]
    r_all = _rows_of(emi[sel], c)
    t_all = tgt[sel] - lo
    cls = (r_all[:, 0] & 1) + 2 * (r_all[:, 1] & 1) + 4 * (r_all[:, 2] & 1)
    order = np.argsort(t_all, kind="stable")
    ts = t_all[order]
    rk = np.arange(ts.size) - np.searchsorted(ts, ts, side="left")
    band = np.empty_like(rk)
    slot = np.empty_like(rk)
    band[order] = rk // c.S
    slot[order] = rk % c.S
    return band, slot, cls, t_all, r_all


def make_plan(inputs, cfg: Cfg):
    """tiles_bc [n_bands, 8]: tiles per (rank-band, parity-class), maxed over
    every (metapath, core) so one compiled layout serves all shards."""
    c = cfg
    nb = 0
    cnts = []
    for mp in range(c.n_mp):
        emi, tgt = _mp_arrays(inputs, mp)
        for k in range(c.n_cores):
            band, slot, cls, t_all, _ = _band_cls(emi, tgt, k, c)
            nb = max(nb, int(band.max()) + 1)
            cnt = np.zeros((int(band.max()) + 1, 8), np.int64)
            np.add.at(cnt, (band, cls), 1)
            cnts.append(cnt)
    tiles_bc = np.zeros((nb, 8), np.int64)
    for cnt in cnts:
        t = (cnt + 127) // 128
        tiles_bc[:t.shape[0]] = np.maximum(tiles_bc[:t.shape[0]], t)
    return tiles_bc, int(tiles_bc.sum())


def _wrap16(vals):
    """[N] values (N % 16 == 0) -> [128, N/16] int16, q7 wrapped layout."""
    v = np.asarray(vals).astype(np.int16).reshape(-1, 16)
    return np.ascontiguousarray(np.tile(v.T, (8, 1)))


def _pack_metapath(emi, tgt, k, c: Cfg):
    """Pack one (metapath, core) shard: band-major, class-sorted in band.

    Row index for the scatter is tgt*S + rank%S (unique within any call,
    since calls never span a band boundary); padding goes to the trash row.
    Returns (emi16 [128, sum(3*nt*8)], tl16 [128, T*8])."""
    band, slot, cls, t_all, r_all = _band_cls(emi, tgt, k, c)
    tiles_bc = c.tiles_bc
    E = c.T * 128
    r_sh = np.zeros((E, 3), np.int64)
    rowi = np.full((E,), c.trash_row, np.int64)
    tpos = 0
    for b in range(tiles_bc.shape[0]):
        for cl in range(8):
            ntiles = int(tiles_bc[b][cl])
            if ntiles == 0:
                continue
            seg = np.nonzero((band == b) & (cls == cl))[0]
            assert seg.size <= ntiles * 128, (b, cl, seg.size, ntiles)
            base = tpos * 128
            r_sh[base:base + seg.size] = r_all[seg]
            dummy = np.array([(cl >> l) & 1 for l in range(3)], np.int64)
            r_sh[base + seg.size:base + ntiles * 128] = dummy
            rowi[base:base + seg.size] = t_all[seg] * c.S + slot[seg]
            tpos += ntiles
    assert tpos == c.T
    pair = r_sh >> 1          # [E, 3] pair-row gather indices (< 20480)
    emi_calls, tl_calls = [], []
    t0 = 0
    for (_, _, nt) in c.calls:
        blk = slice(t0 * 128, (t0 + nt) * 128)
        stream = np.concatenate([pair[blk, l] for l in range(3)])
        emi_calls.append(_wrap16(stream))
        tl_calls.append(_wrap16(rowi[blk]))
        t0 += nt
    assert t0 == c.T
    return (np.concatenate(emi_calls, axis=1),
            np.concatenate(tl_calls, axis=1))


def prepare(inputs, cfg: Cfg):
    c = cfg
    tbc, T = make_plan(inputs, cfg)
    c.tiles_bc = tbc
    c.T = T

    f0, f1 = np.asarray(inputs["feats0"]), np.asarray(inputs["feats1"])
    feats_all = np.concatenate([f0, f1], axis=0)
    attn4 = np.stack([np.asarray(inputs["attn_user"][p]).reshape(-1) for p in range(2)] +
                     [np.asarray(inputs["attn_item"][p]).reshape(-1) for p in range(2)])
    rv = np.asarray(inputs["r_vec"])[0].reshape(-1).astype(np.float32)

    in_maps = []
    for k in range(c.n_cores):
        m = {}
        lo_n = k * c.nodes_real
        fs = feats_all[lo_n:lo_n + c.nodes_real]
        pad = c.nodes_core - c.nodes_real
        if pad:
            fs = np.concatenate([fs, np.zeros((pad, c.F0), np.float32)], axis=0)
        m["feats"] = np.ascontiguousarray(fs, np.float32)
        tw = "0" if lo_n < f0.shape[0] else "1"
        vals = {
            **{nm: np.asarray(inputs[f"tower{tw}_{nm}"], np.float32)
               for nm in ("pw", "pb", "w2", "b2", "g", "be")},
            "rvec": rv, "attn": attn4.astype(np.float32),
            "suw1": inputs["su_w1"], "sub1": inputs["su_b1"],
            "suw2": inputs["su_w2"], "siw1": inputs["si_w1"],
            "sib1": inputs["si_b1"], "siw2": inputs["si_w2"],
            "cw1": inputs["cw1"], "cb1": inputs["cb1"],
            "cw2t": np.asarray(inputs["cw2"], np.float32).T,
        }
        blob = np.zeros(WB_TOTAL, np.float32)
        for nm, sz in _WB_FIELDS:
            v = np.asarray(vals[nm], np.float32).reshape(-1)
            assert v.size == sz, (nm, v.size, sz)
            blob[WB_OFF[nm]:WB_OFF[nm] + sz] = v
        m["wblob"] = blob
        emi_l, tl_l = [], []
        for mp in range(c.n_mp):
            emi, tgt = _mp_arrays(inputs, mp)
            e16, t16 = _pack_metapath(emi, tgt, k, c)
            emi_l.append(e16)
            tl_l.append(t16)
        m["emi16"] = np.concatenate(emi_l, axis=0)
        m["tl16"] = np.concatenate(tl_l, axis=0)
        in_maps.append(m)
    return in_maps


# ---------------------------------------------------------------------------
# PJRT SPMD runner (axon path)
# ---------------------------------------------------------------------------


class SpmdRunner:
    def __init__(self, nc, n_cores: int):
        import jax
        from jax.sharding import Mesh, PartitionSpec, NamedSharding
        from jax.experimental.shard_map import shard_map
        from concourse.bass2jax import (
            _bass_exec_p, install_neuronx_cc_hook, partition_id_tensor)

        self.jax = jax
        install_neuronx_cc_hook()
        self.nc = nc
        self.n_cores = n_cores
        partition_name = nc.partition_id_tensor.name if nc.partition_id_tensor else None
        in_names, out_names, out_avals, zero_outs = [], [], [], []
        for alloc in nc.m.functions[0].allocations:
            if not isinstance(alloc, mybir.MemoryLocationSet):
                continue
            name = alloc.memorylocations[0].name
            if alloc.kind == "ExternalInput":
                if name != partition_name:
                    in_names.append(name)
            elif alloc.kind == "ExternalOutput":
                out_names.append(name)
                shape = tuple(alloc.tensor_shape)
                dtype = mybir.dt.np(alloc.dtype)
                out_avals.append(jax.core.ShapedArray(shape, dtype))
                zero_outs.append(np.zeros(shape, dtype))
        self.dbg_name = nc.dbg_addr.name if nc.dbg_addr is not None else None
        n_params = len(in_names)
        in_names = in_names + out_names
        if partition_name is not None:
            in_names.append(partition_name)
        self.in_names, self.out_names = in_names, out_names
        self.n_params, self.out_avals, self.zero_outs = n_params, out_avals, zero_outs

        def _body(*args):
            operands = list(args)
            if partition_name is not None:
                operands.append(partition_id_tensor())
            outs = _bass_exec_p.bind(
                *operands,
                out_avals=tuple(out_avals),
                in_names=tuple(in_names),
                out_names=tuple(out_names),
                lowering_input_output_aliases=(),
                sim_require_finite=True,
                sim_require_nnan=True,
                nc=nc,
            )
            return tuple(outs)

        devices = jax.devices()[:n_cores]
        assert len(devices) == n_cores
        self.mesh = Mesh(np.asarray(devices), ("core",))
        donate = tuple(range(n_params, n_params + len(out_names)))
        in_specs = (PartitionSpec("core"),) * (n_params + len(out_names))
        out_specs = (PartitionSpec("core"),) * len(out_names)
        self.sharded = jax.jit(
            shard_map(_body, mesh=self.mesh, in_specs=in_specs,
                      out_specs=out_specs, check_rep=False),
            donate_argnums=donate, keep_unused=True)
        self.sharding = NamedSharding(self.mesh, PartitionSpec("core"))

    def stage_inputs(self, in_maps):
        jax = self.jax
        if self.dbg_name is not None:
            in_maps = [{**m, self.dbg_name: np.zeros((1, 2), np.uint32)}
                       for m in in_maps]
        staged = []
        for i in range(self.n_params):
            name = self.in_names[i]
            arr = np.concatenate([np.asarray(m[name]) for m in in_maps], axis=0)
            staged.append(jax.device_put(arr, self.sharding))
        jax.block_until_ready(staged)
        self.staged = staged

    def _zeros(self):
        jax = self.jax
        zs = [jax.device_put(
            np.zeros((self.n_cores * z.shape[0], *z.shape[1:]), z.dtype),
            self.sharding) for z in self.zero_outs]
        jax.block_until_ready(zs)
        return zs

    def run(self):
        jax = self.jax
        outs = self.sharded(*self.staged, *self._zeros())
        jax.block_until_ready(outs)
        return [
            {name: np.asarray(outs[i]).reshape(self.n_cores, *self.out_avals[i].shape)[k]
             for i, name in enumerate(self.out_names)}
            for k in range(self.n_cores)
        ]

    def bench(self, iters=20, warmup=3):
        import time
        jax = self.jax
        times = []
        for it in range(warmup + iters):
            zs = self._zeros()
            t0 = time.perf_counter()
            outs = self.sharded(*self.staged, *zs)
            jax.block_until_ready(outs)
            dt = time.perf_counter() - t0
            if it >= warmup:
                times.append(dt)
            del outs
        times = np.array(times)
        return {"min_s": float(times.min()), "med_s": float(np.median(times)),
                "mean_s": float(times.mean()), "n": iters}


_CACHE = {}


def kernel(**inputs) -> np.ndarray:
    cfg = Cfg()
    in_maps = prepare(inputs, cfg)
    key = (cfg.T, cfg.tiles_bc.tobytes())
    if key not in _CACHE:
        nc = build_program(cfg)
        _CACHE[key] = (nc, SpmdRunner(nc, cfg.n_cores))
    nc, runner = _CACHE[key]
    runner.stage_inputs(in_maps)
    res = runner.run()
    out = np.empty((cfg.B, 2), np.float32)
    for k in range(cfg.n_cores):
        out[k * cfg.B_loc:(k + 1) * cfg.B_loc] = res[k]["out"]
    return out


# revision 68
# speedup vs baseline: 27.2184x; 11.0216x over previous
"""Trainium2 Bass kernel for MAGNN link prediction (nn_MAGNN_lp).

Sharding: the B=8192 targets are split across 8 cores (1024 each) and each
core owns the metapath instances whose target falls in its range, so the
segment softmax/sum is core-local. Node towers are sharded by node rows
(5000/core, padded to 5120); the projected node table is AllGathered in
fp16 and stored as PAIRED rows [20480, 128] so every dma_gather element is
the 256B hardware minimum with no lo/hi split (idx = row>>1 fits int16;
instances are class-sorted by the parity triple of their 3 node rows so the
64-column slice offset of each tile run is compile-time).

Per chunk each core gathers all 3 metapath positions in ONE call, computes
(on fp16 DVE ops) u = ed0+ed2, the attention logit via the rotation pushed
into the attention vector (e = u.a + ed1.rot_adj(a)), exp(e-6) (global exp
shift; cancels in the softmax), and dma_scatter_adds the 136-float payload
[w*u | w*ed1 | w] into a per-target DRAM accumulator (trailing trash row
absorbs padding instances). The per-target rotation of sum(w*ed1) is applied
once per target in the head (rotation is linear), followed by normalize,
ELU, semantic attention (one tiny AllReduce), the product MLP and softmax.
Host work is integer packing of index tensors only.
"""
import numpy as np

import concourse.bass as bass
import concourse.mybir as mybir
import concourse.tile as tile
from concourse import bacc
from concourse.masks import make_identity
from dataclasses import dataclass

F32 = mybir.dt.float32
F16 = mybir.dt.float16
I16 = mybir.dt.int16
I32 = mybir.dt.int32
AF = mybir.ActivationFunctionType
ALU = mybir.AluOpType
PSUM = "PSUM"

ESHIFT = 6.0           # exp(e - ESHIFT); cancels in softmax ratio
EPS_S = 1e-9 * float(np.exp(-ESHIFT))


@dataclass
class Cfg:
    n_cores: int = 8
    B: int = 8192
    HID: int = 64
    H: int = 8
    D: int = 8
    F0: int = 512
    AV: int = 128
    CH: int = 128
    nodes_real: int = 5000      # real nodes per core
    nodes_core: int = 5120      # padded to 128
    Tc: int = 28                # max tiles per gather/scatter call
    S: int = 8                  # accumulator slots per target (HW scatter-add
                                # loses colliding updates; slots make every
                                # row within one call unique)
    T: int = 200                # tiles per metapath per core (plan sets)
    n_mp: int = 4
    gelu: bool = True           # False: Tanh stand-in (CoreSim lacks Gelu)
    dbg: bool = False
    tiles_bc: np.ndarray | None = None   # [n_bands, 8] maxed over mp, core

    @property
    def B_loc(self):
        return self.B // self.n_cores

    @property
    def n_rows(self):
        return self.nodes_core * self.n_cores    # 40960

    @property
    def n_pairs(self):
        return self.n_rows // 2                  # 20480

    @property
    def node_tiles(self):
        return self.nodes_core // 128            # 40

    @property
    def E_loc(self):
        return self.T * 128

    @property
    def kF(self):
        return self.F0 // 128

    @property
    def b_tiles(self):
        return self.B_loc // 128

    @property
    def trash_row(self):
        return self.B_loc * self.S

    @property
    def acc_rows(self):
        return self.B_loc * self.S + 128         # trailing trash rows

    @property
    def acc_step(self):
        return 192                               # 768B row stride (256B mult)

    @property
    def calls(self):
        """[(band, tile_off_in_band, ntiles)] — call windows, band-aligned."""
        out = []
        for b in range(self.tiles_bc.shape[0]):
            nb = int(self.tiles_bc[b].sum())
            off = 0
            while off < nb:
                take = min(self.Tc, nb - off)
                out.append((b, off, take))
                off += take
        return out


PAYW = 136     # payload floats per instance: w*u(64) | w*ed1(64) | w(8)

# flat layout of the packed small-weights blob (all f32)
_WB_FIELDS = [
    ("pw", 512 * 64), ("pb", 64), ("w2", 64 * 64), ("b2", 64), ("g", 64),
    ("be", 64), ("rvec", 64), ("attn", 4 * 64),
    ("suw1", 64 * 128), ("sub1", 128), ("suw2", 128),
    ("siw1", 64 * 128), ("sib1", 128), ("siw2", 128),
    ("cw1", 64 * 128), ("cb1", 128), ("cw2t", 2 * 128),
]
WB_OFF = {}
_o = 0
for _nm, _sz in _WB_FIELDS:
    WB_OFF[_nm] = _o
    _o += _sz
WB_TOTAL = _o


def _call_runs(tiles_bc, calls):
    """runs per call: [(toff_in_call, ntiles, cls)] from band class layout."""
    out = []
    for (b, off, nt) in calls:
        runs, t = [], 0
        for cls in range(8):
            n = int(tiles_bc[b][cls])
            a0, a1 = max(off, t), min(off + nt, t + n)
            if a0 < a1:
                runs.append((a0 - off, a1 - a0, cls))
            t += n
        out.append(runs)
    return out


def _ap_with(ap, offset_delta, tail_dims):
    """Copy an AP, keeping its partition dim, replacing trailing free dims."""
    return bass.AP(ap.tensor, ap.offset + offset_delta,
                   [list(ap.ap[0])] + [list(d) for d in tail_dims])


def build_program(cfg: Cfg):
    c = cfg
    assert c.tiles_bc is not None
    nc = bacc.Bacc("TRN2", target_bir_lowering=False, debug=False,
                   num_devices=c.n_cores, num_swdge_queues=2)

    def di(name, shape, dtype=F32):
        return nc.dram_tensor(name, list(shape), dtype, kind="ExternalInput")

    T8 = c.T * 8
    feats = di("feats", (c.nodes_core, c.F0))
    wblob = di("wblob", (WB_TOTAL,))
    emi16 = di("emi16", (c.n_mp * 128, 3 * T8), I16)
    tl16 = di("tl16", (c.n_mp * 128, T8), I16)

    def wb(nm, dims, extra=0):
        return bass.AP(wblob.ap().tensor, WB_OFF[nm] + extra,
                       [list(d) for d in dims])

    def wbrow(nm, n, extra=0):
        return wb(nm, [[n, 1], [1, n]], extra)

    outd = nc.dram_tensor("out", [c.B_loc, 2], F32, kind="ExternalOutput")
    if c.dbg:
        dbg_acc = nc.dram_tensor("dbg_acc", [c.B_loc, PAYW], F32,
                                 kind="ExternalOutput")
        dbg_tab = nc.dram_tensor("dbg_tab", [2048, 64], F16,
                                 kind="ExternalOutput")
        dbg_ed = nc.dram_tensor("dbg_ed", [128, 384], F16,
                                kind="ExternalOutput")

    HID, H, D, Tc = c.HID, c.H, c.D, c.Tc

    with tile.TileContext(nc) as tc:
        with (
            tc.tile_pool(name="const", bufs=1) as kpool,
            tc.tile_pool(name="dram", bufs=1, space="DRAM") as dpool,
        ):
            pk_ctx = tc.tile_pool(name="ps_const", bufs=1, space="PSUM")
            pkpool = pk_ctx.__enter__()
            # ---------- constants ----------
            id128 = kpool.tile([128, 128], F32, tag="id128")
            make_identity(nc, id128[:])
            ones1 = kpool.tile([1, 128], F32, tag="ones1")
            nc.vector.memset(ones1[:], 1.0)
            onescol = kpool.tile([128, 1], F32, tag="onescol")
            nc.vector.memset(onescol[:], 1.0)
            epscol = kpool.tile([128, 1], F32, tag="epscol")
            nc.vector.memset(epscol[:], 1e-5)
            shiftcol = kpool.tile([128, 1], F32, tag="shiftcol")
            nc.vector.memset(shiftcol[:], -ESHIFT)
            zacc = kpool.tile([128, 780], F32, tag="zacc")
            nc.vector.memset(zacc[:], 0.0)

            def rep_row(dram_vec, n, scale=None, tag=None, dtype=F32):
                row = kpool.tile([1, n], F32, tag=f"{tag}_row")
                nc.sync.dma_start(row[:], dram_vec)
                return rep_from_row(row[:], n, tag, scale=scale, dtype=dtype)

            def rep_from_row(row_ap, n, tag, scale=None, dtype=F32):
                ps = pkpool.tile([128, 512], F32, space=PSUM, tag="reppsum")
                nc.tensor.matmul(out=ps[:, :n], lhsT=ones1[:], rhs=row_ap,
                                 start=True, stop=True)
                rep = kpool.tile([128, n], dtype, tag=tag)
                if scale is None:
                    nc.vector.tensor_copy(rep[:], ps[:, :n])
                else:
                    nc.vector.tensor_scalar_mul(rep[:], ps[:, :n], scale)
                return rep

            PBrep = rep_row(wbrow("pb", HID), HID, tag="PBrep")
            B2rep = rep_row(wbrow("b2", HID), HID, tag="B2rep")
            G3rep = rep_row(wbrow("g", HID), HID, scale=1.0 / 3.0, tag="G3rep")
            BE3rep = rep_row(wbrow("be", HID), HID, scale=1.0 / 3.0, tag="BE3rep")
            SUB1rep = rep_row(wbrow("sub1", c.AV), c.AV, tag="SUB1rep")
            SIB1rep = rep_row(wbrow("sib1", c.AV), c.AV, tag="SIB1rep")
            SUW2rep = rep_row(wbrow("suw2", c.AV), c.AV, tag="SUW2rep")
            SIW2rep = rep_row(wbrow("siw2", c.AV), c.AV, tag="SIW2rep")
            CB1rep = rep_row(wbrow("cb1", c.CH), c.CH, tag="CB1rep")
            CW20rep = rep_row(wbrow("cw2t", c.CH), c.CH, tag="CW20rep")
            CW21rep = rep_row(wbrow("cw2t", c.CH, extra=c.CH), c.CH, tag="CW21rep")

            # ---------- rotation constants (normalize r on device) ----------
            # (same construction as before: crrow = Re(r) per feature,
            # c2urow/c2irow = +/- Im(r) with per-parity sign)
            rcol = kpool.tile([HID, 1], F32, tag="rcol")
            nc.sync.dma_start(rcol[:], wb("rvec", [[1, HID], [1, 1]]))
            idh = kpool.tile([HID, HID], F32, tag="idh")
            make_identity(nc, idh[:])
            Sp = kpool.tile([HID, HID], F32, tag="Sp")
            nc.vector.memset(Sp[:], 0.0)
            nc.vector.tensor_copy(Sp[:, 1:HID], idh[:, 0:HID - 1])
            Sm = kpool.tile([HID, HID], F32, tag="Sm")
            nc.vector.memset(Sm[:], 0.0)
            nc.vector.tensor_copy(Sm[:, 0:HID - 1], idh[:, 1:HID])
            pidx = kpool.tile([HID, 1], I32, tag="pidx")
            nc.gpsimd.iota(pidx[:], pattern=[[0, 1]], base=0, channel_multiplier=1)
            podd_i = kpool.tile([HID, 1], I32, tag="podd_i")
            nc.vector.tensor_scalar(podd_i[:], pidx[:], 1, None, ALU.bitwise_and)
            podd = kpool.tile([HID, 1], F32, tag="podd")
            nc.vector.tensor_copy(podd[:], podd_i[:])
            peven = kpool.tile([HID, 1], F32, tag="peven")
            nc.vector.tensor_scalar(peven[:], podd[:], -1.0, -1.0, ALU.add, ALU.mult)
            Spe = kpool.tile([HID, HID], F32, tag="Spe")
            nc.vector.tensor_scalar_mul(Spe[:], Sp[:], peven[:])
            Smo = kpool.tile([HID, HID], F32, tag="Smo")
            nc.vector.tensor_scalar_mul(Smo[:], Sm[:], podd[:])
            Ie = kpool.tile([HID, HID], F32, tag="Ie")
            nc.vector.tensor_scalar_mul(Ie[:], idh[:], peven[:])
            Io = kpool.tile([HID, HID], F32, tag="Io")
            nc.vector.tensor_scalar_mul(Io[:], idh[:], podd[:])
            M2 = kpool.tile([HID, HID], F32, tag="M2")
            nc.vector.tensor_tensor(M2[:], idh[:], Spe[:], ALU.add)
            nc.vector.tensor_tensor(M2[:], M2[:], Smo[:], ALU.add)
            Me = kpool.tile([HID, HID], F32, tag="Me")
            nc.vector.tensor_tensor(Me[:], Ie[:], Spe[:], ALU.add)
            Mo = kpool.tile([HID, HID], F32, tag="Mo")
            nc.vector.tensor_tensor(Mo[:], Io[:], Smo[:], ALU.add)
            sqc = kpool.tile([HID, 1], F32, tag="sqc")
            nc.vector.tensor_tensor(sqc[:], rcol[:], rcol[:], ALU.mult)
            n2 = pkpool.tile([HID, 1], F32, space=PSUM, tag="n2")
            nc.tensor.matmul(out=n2[:], lhsT=M2[:], rhs=sqc[:], start=True, stop=True)
            nrm = kpool.tile([HID, 1], F32, tag="nrm")
            nc.scalar.activation(nrm[:], n2[:], AF.Sqrt)
            invn = kpool.tile([HID, 1], F32, tag="invn")
            nc.vector.reciprocal(invn[:], nrm[:])
            rn = kpool.tile([HID, 1], F32, tag="rn")
            nc.vector.tensor_scalar_mul(rn[:], rcol[:], invn[:])
            cr2 = pkpool.tile([HID, 1], F32, space=PSUM, tag="cr2")
            nc.tensor.matmul(out=cr2[:], lhsT=Me[:], rhs=rn[:], start=True, stop=True)
            ci2 = pkpool.tile([HID, 1], F32, space=PSUM, tag="ci2")
            nc.tensor.matmul(out=ci2[:], lhsT=Mo[:], rhs=rn[:], start=True, stop=True)
            cr2s = kpool.tile([HID, 1], F32, tag="cr2s")
            nc.vector.tensor_copy(cr2s[:], cr2[:])
            ci2s = kpool.tile([HID, 1], F32, tag="ci2s")
            nc.vector.tensor_copy(ci2s[:], ci2[:])
            crrow_ps = pkpool.tile([1, HID], F32, space=PSUM, tag="crrow_ps")
            nc.tensor.matmul(out=crrow_ps[:], lhsT=cr2s[:], rhs=idh[:], start=True, stop=True)
            crrow = kpool.tile([1, HID], F32, tag="crrow")
            nc.vector.tensor_copy(crrow[:], crrow_ps[:])
            cirow_ps = pkpool.tile([1, HID], F32, space=PSUM, tag="cirow_ps")
            nc.tensor.matmul(out=cirow_ps[:], lhsT=ci2s[:], rhs=idh[:], start=True, stop=True)
            cirow = kpool.tile([1, HID], F32, tag="cirow")
            nc.vector.tensor_copy(cirow[:], cirow_ps[:])
            fidx = kpool.tile([1, HID], I32, tag="fidx")
            nc.gpsimd.iota(fidx[:], pattern=[[1, HID]], base=0, channel_multiplier=0)
            fodd_i = kpool.tile([1, HID], I32, tag="fodd_i")
            nc.vector.tensor_scalar(fodd_i[:], fidx[:], 1, None, ALU.bitwise_and)
            fsign = kpool.tile([1, HID], F32, tag="fsign")
            nc.vector.tensor_copy(fsign[:], fodd_i[:])
            nc.vector.tensor_scalar(fsign[:], fsign[:], -2.0, 1.0, ALU.mult, ALU.add)
            c2u = kpool.tile([1, HID], F32, tag="c2u")
            c2i = kpool.tile([1, HID], F32, tag="c2i")
            c2row = [c2u, c2i]
            nc.vector.tensor_tensor(c2row[0][:], cirow[:], fsign[:], ALU.mult)
            nc.vector.tensor_scalar_mul(c2row[1][:], c2row[0][:], -1.0)
            C1rep = rep_from_row(crrow[:], HID, "C1rep")
            C2rep = [rep_from_row(c2row[0][:], HID, "C2urep"),
                     rep_from_row(c2row[1][:], HID, "C2irep")]

            # attention rows (fp16 replicated) + rotation-adjoint rows:
            # a2_d = C1_d * a_d + C2_{d^1} * a_{d^1}
            ATT16, A2_16 = [], []
            for mp in range(c.n_mp):
                side = 0 if mp < 2 else 1
                arow = kpool.tile([1, HID], F32, tag=f"arow{mp}")
                nc.sync.dma_start(arow[:], wbrow("attn", HID, extra=mp * HID))
                m = kpool.tile([1, HID], F32, tag=f"m{mp}")
                nc.vector.tensor_tensor(m[:], c2row[side][:], arow[:], ALU.mult)
                a2 = kpool.tile([1, HID], F32, tag=f"a2_{mp}")
                nc.vector.tensor_tensor(a2[:], crrow[:], arow[:], ALU.mult)
                mswap = _ap_with(m[:], 1, [[2, HID // 2], [-1, 2]])
                nc.vector.tensor_tensor(a2[:], a2[:], mswap, ALU.add)
                ATT16.append(rep_from_row(arow[:], HID, f"AT16_{mp}", dtype=F16))
                A2_16.append(rep_from_row(a2[:], HID, f"A216_{mp}", dtype=F16))

            pwsb = kpool.tile([128, c.kF, HID], F32, tag="pwsb")
            nc.sync.dma_start(
                pwsb[:], wb("pw", [[HID, 128], [128 * HID, c.kF], [1, HID]]))
            w2sb = kpool.tile([HID, HID], F32, tag="w2sb")
            nc.sync.dma_start(w2sb[:], wb("w2", [[HID, HID], [1, HID]]))
            suw1sb = kpool.tile([HID, c.AV], F32, tag="suw1sb")
            nc.sync.dma_start(suw1sb[:], wb("suw1", [[c.AV, HID], [1, c.AV]]))
            siw1sb = kpool.tile([HID, c.AV], F32, tag="siw1sb")
            nc.sync.dma_start(siw1sb[:], wb("siw1", [[c.AV, HID], [1, c.AV]]))
            cw1sb = kpool.tile([HID, c.CH], F32, tag="cw1sb")
            nc.sync.dma_start(cw1sb[:], wb("cw1", [[c.CH, HID], [1, c.CH]]))

            pk_ctx.__exit__(None, None, None)

            # ---------- dram tiles ----------
            tower_t = dpool.tile([c.nodes_core, HID], F16, tag="tower")
            table_t = dpool.tile([c.n_rows, HID], F16, tag="table")
            accs = [dpool.tile([c.acc_rows, c.acc_step], F32, tag=f"acc{mp}",
                               name=f"acc{mp}")
                    for mp in range(c.n_mp)]
            # acc_rows*acc_step == 128*12480; zero in 8 contiguous strips
            assert c.acc_rows * c.acc_step == 128 * 12480
            for mp in range(c.n_mp):
                for j in range(16):
                    dst = bass.AP(accs[mp][:].tensor,
                                  accs[mp][:].offset + j * 780,
                                  [[12480, 128], [1, 780]])
                    nc.sync.dma_start(dst, zacc[:])

            # ---------- tower (two passes to avoid act-table thrash) ----------
            nt = c.node_tiles
            with (
                tc.tile_pool(name="tw_x", bufs=2) as xpool,
                tc.tile_pool(name="tw_ps", bufs=2, space="PSUM") as tpspool,
                tc.tile_pool(name="tw_s", bufs=2) as tspool,
                tc.tile_pool(name="tw_keep", bufs=1) as tkpool,
            ):
                Z = tkpool.tile([128, nt, HID], F32, tag="Z")
                HH = tkpool.tile([128, nt, HID], F32, tag="HH")
                YC = tkpool.tile([128, nt, HID], F32, tag="YC")
                VV = tkpool.tile([128, nt], F32, tag="VV")
                for j in range(nt):
                    xt = xpool.tile([128, c.F0], F32, tag="xt")
                    nc.sync.dma_start(xt[:], feats.ap()[j * 128:(j + 1) * 128, :])
                    xT = xpool.tile([128, c.kF, 128], F32, tag="xT")
                    for kk in range(c.kF):
                        pst = tpspool.tile([128, 128], F32, space=PSUM, tag="pst")
                        nc.tensor.transpose(pst[:], xt[:, kk * 128:(kk + 1) * 128], id128[:])
                        nc.scalar.activation(xT[:, kk, :], pst[:], AF.Copy)
                    z = tpspool.tile([128, HID], F32, space=PSUM, tag="z")
                    for kk in range(c.kF):
                        nc.tensor.matmul(out=z[:], lhsT=xT[:, kk, :], rhs=pwsb[:, kk, :],
                                         start=(kk == 0), stop=(kk == c.kF - 1))
                    nc.vector.tensor_tensor(Z[:, j, :], z[:], PBrep[:], ALU.add)
                # one activation call for the whole tower
                nc.scalar.activation(HH[:].rearrange("p a b -> p (a b)"),
                                     Z[:].rearrange("p a b -> p (a b)"),
                                     AF.Gelu if c.gelu else AF.Tanh)
                for j in range(nt):
                    hT_ps = tpspool.tile([HID, 128], F32, space=PSUM, tag="hT_ps")
                    nc.tensor.transpose(hT_ps[:], HH[:, j, :], id128[:])
                    hT = tspool.tile([HID, 128], F32, tag="hT")
                    nc.scalar.activation(hT[:], hT_ps[:], AF.Copy)
                    y = tpspool.tile([128, HID], F32, space=PSUM, tag="y")
                    nc.tensor.matmul(out=y[:], lhsT=hT[:], rhs=w2sb[:], start=True, stop=True)
                    ys = tspool.tile([128, HID], F32, tag="ys")
                    nc.vector.tensor_tensor(ys[:], y[:], B2rep[:], ALU.add)
                    nc.vector.tensor_tensor(ys[:], ys[:], Z[:, j, :], ALU.add)
                    mu = tspool.tile([128, 1], F32, tag="mu")
                    nc.vector.tensor_reduce(mu[:], ys[:], mybir.AxisListType.X, ALU.add)
                    nc.vector.tensor_scalar_mul(mu[:], mu[:], 1.0 / HID)
                    nc.vector.tensor_scalar(YC[:, j, :], ys[:], mu[:], None, ALU.subtract)
                    sq = tspool.tile([128, HID], F32, tag="sq")
                    nc.vector.tensor_tensor(sq[:], YC[:, j, :], YC[:, j, :], ALU.mult)
                    nc.vector.tensor_reduce(VV[:, j:j + 1], sq[:], mybir.AxisListType.X, ALU.add)
                sdv = tspool.tile([128, nt], F32, tag="sdv")
                nc.scalar.activation(sdv[:], VV[:], AF.Sqrt, bias=epscol[:],
                                     scale=1.0 / HID)
                inv = tspool.tile([128, nt], F32, tag="inv")
                nc.vector.reciprocal(inv[:], sdv[:])
                t1 = tkpool.tile([128, nt, HID], F32, tag="t1")
                invb = _ap_with(inv[:], 0, [[1, nt], [0, HID]])
                nc.vector.tensor_tensor(t1[:], YC[:], invb, ALU.mult)
                g3b = _ap_with(G3rep[:], 0, [[0, nt], [1, HID]])
                be3b = _ap_with(BE3rep[:], 0, [[0, nt], [1, HID]])
                TS = tkpool.tile([128, nt, HID], F16, tag="TS")
                nc.vector.tensor_tensor(TS[:], t1[:], g3b, ALU.mult)
                nc.vector.tensor_tensor(TS[:], TS[:], be3b, ALU.add)
                # SBUF [128, nt, HID] -> DRAM rows (j*128+p)
                dst = bass.AP(tower_t[:].tensor, tower_t[:].offset,
                              [[HID, 128], [128 * HID, nt], [1, HID]])
                nc.sync.dma_start(dst, TS[:])

            nc.gpsimd.collective_compute(
                "AllGather", ALU.bypass,
                replica_groups=[list(range(c.n_cores))],
                ins=[tower_t.opt()], outs=[table_t.opt()],
            )
            # paired-row view for gathers: [n_pairs, 128] fp16
            table_pairs = bass.AP(table_t[:].tensor, table_t[:].offset,
                                  [[128, c.n_pairs], [1, 128]])

            # ---------- metapath chunks: gather, logits, scatter-add ----------
            calls = c.calls
            runs_all = _call_runs(c.tiles_bc, calls)
            # per-call column offsets into the packed emi16 / tl16 streams
            emi_off, tl_off = [0], [0]
            for (_, _, nt) in calls:
                emi_off.append(emi_off[-1] + 3 * nt * 8)
                tl_off.append(tl_off[-1] + nt * 8)
            with (
                tc.tile_pool(name="mp_idx", bufs=2) as ipool,
                tc.tile_pool(name="mp_ed", bufs=3) as edpool,
                tc.tile_pool(name="mp_row", bufs=2) as rowpool,
                tc.tile_pool(name="mp_tmp", bufs=1) as mtpool,
                tc.tile_pool(name="mp_tl", bufs=1) as tlpool,
                tc.tile_pool(name="hd_acc", bufs=2) as apool,
                tc.tile_pool(name="hd_b", bufs=1) as bhpool,
                tc.tile_pool(name="hd_s", bufs=3) as hpool,
                tc.tile_pool(name="hd_ps", bufs=1, space="PSUM") as hpspool,
                tc.tile_pool(name="hd_keep", bufs=1) as keep,
            ):
                outs_all = keep.tile([128, c.n_mp, c.b_tiles, HID], F32, tag="outs_all")
                acc4 = keep.tile([1, c.n_mp], F32, tag="acc4")
                nc.vector.memset(acc4[:], 0.0)

                emi_sbs, tl_sbs, eds = {}, {}, {}
                gsem = nc.alloc_semaphore("gsem")
                ssem = nc.alloc_semaphore("ssem")

                def emit_gather(mp, ci, direct=True):
                    _, _, nt = calls[ci]
                    ed = edpool.tile([128, 3 * Tc, 128], F16, tag="ed")
                    eds[(mp, ci)] = ed
                    nc.gpsimd.dma_gather(
                        out_ap=ed[:, 0:3 * nt, :], in_ap=table_pairs,
                        idxs_ap=emi_sbs[mp][:, emi_off[ci]:emi_off[ci + 1]],
                        num_idxs=3 * nt * 128, num_idxs_reg=3 * nt * 128,
                        elem_size=128, single_packet=False)

                def emit_compute_scatter(mp, ci):
                    ed = eds.pop((mp, ci))
                    _, _, nt = calls[ci]
                    runs = runs_all[ci]
                    u = mtpool.tile([128, Tc, HID], F16, tag="u")
                    v = mtpool.tile([128, Tc, HID], F16, tag="v")
                    t2 = mtpool.tile([128, Tc, HID], F16, tag="t2")
                    for (t0, ntk, cls) in runs:
                        p0, p1, p2 = cls & 1, (cls >> 1) & 1, (cls >> 2) & 1
                        e0 = ed[:, t0:t0 + ntk, p0 * HID:p0 * HID + HID]
                        e1 = ed[:, nt + t0:nt + t0 + ntk, p1 * HID:p1 * HID + HID]
                        e2 = ed[:, 2 * nt + t0:2 * nt + t0 + ntk, p2 * HID:p2 * HID + HID]
                        nc.vector.tensor_tensor(u[:, t0:t0 + ntk, :], e0, e2, ALU.add)
                        a2b = _ap_with(A2_16[mp][:], 0, [[0, ntk], [1, HID]])
                        nc.vector.tensor_tensor(t2[:, t0:t0 + ntk, :], e1, a2b, ALU.mult)
                    ab = _ap_with(ATT16[mp][:], 0, [[0, nt], [1, HID]])
                    nc.vector.tensor_tensor(v[:, 0:nt, :], u[:, 0:nt, :], ab, ALU.mult)
                    nc.vector.tensor_tensor(v[:, 0:nt, :], v[:, 0:nt, :],
                                            t2[:, 0:nt, :], ALU.add)
                    e32 = mtpool.tile([128, Tc, H], F32, tag="e32")
                    nc.vector.tensor_reduce(
                        e32[:, 0:nt, :],
                        v[:, 0:nt, :].rearrange("p t (h d) -> p t h d", d=D),
                        mybir.AxisListType.X, ALU.add)
                    el = mtpool.tile([128, Tc, H], F32, tag="el")
                    nc.vector.tensor_scalar_mul(el[:, 0:nt, :], e32[:, 0:nt, :], 0.01)
                    nc.vector.tensor_tensor(el[:, 0:nt, :], el[:, 0:nt, :],
                                            e32[:, 0:nt, :], ALU.max)
                    w16 = mtpool.tile([128, Tc, H], F16, tag="w16")
                    nc.scalar.activation(w16[:, 0:nt, :], el[:, 0:nt, :], AF.Exp,
                                         bias=shiftcol[:])
                    rows = rowpool.tile([128, Tc, PAYW], F32, tag="rows")
                    wb = _ap_with(w16[:], 0, [[H, nt], [1, H], [0, D]])
                    nc.vector.tensor_tensor(rows[:, 0:nt, 0:HID], u[:, 0:nt, :],
                                            wb, ALU.mult)
                    nc.scalar.activation(rows[:, 0:nt, 2 * HID:PAYW],
                                         w16[:, 0:nt, :], AF.Copy)
                    for (t0, ntk, cls) in runs:
                        p1 = (cls >> 1) & 1
                        e1 = ed[:, nt + t0:nt + t0 + ntk, p1 * HID:p1 * HID + HID]
                        wbr = _ap_with(w16[:], t0 * H, [[H, ntk], [1, H], [0, D]])
                        nc.vector.tensor_tensor(rows[:, t0:t0 + ntk, HID:2 * HID],
                                                e1, wbr, ALU.mult)
                    acc_ap = bass.AP(accs[mp][:].tensor, accs[mp][:].offset,
                                     [[c.acc_step, c.acc_rows], [1, PAYW]])
                    nc.gpsimd.dma_scatter_add(
                        out_ap=acc_ap, in_ap=rows[:, 0:nt, :],
                        idxs_ap=tl_sbs[mp][:, tl_off[ci]:tl_off[ci + 1]],
                        num_idxs=nt * 128, num_idxs_reg=nt * 128,
                        elem_size=PAYW, elem_step=c.acc_step,
                        single_packet=False)

                def emit_head(mp):
                    side = 0 if mp < 2 else 1
                    w1sb = suw1sb if mp < 2 else siw1sb
                    b1rep = SUB1rep if mp < 2 else SIB1rep
                    w2rep = SUW2rep if mp < 2 else SIW2rep
                    bts = c.b_tiles
                    f1 = apool.tile([128, bts, PAYW], F32, tag="f1")
                    for bt in range(bts):
                        acc_sb = apool.tile([128, c.S, PAYW], F32, tag="acc_sb")
                        src = bass.AP(accs[mp][:].tensor,
                                      accs[mp][:].offset
                                      + bt * 128 * c.S * c.acc_step,
                                      [[c.S * c.acc_step, 128],
                                       [c.acc_step, c.S], [1, PAYW]])
                        nc.sync.dma_start(acc_sb[:], src)
                        f4 = apool.tile([128, 4, PAYW], F32, tag="f4",
                                        bufs=1)
                        nc.vector.tensor_tensor(f4[:], acc_sb[:, 0:4, :],
                                                acc_sb[:, 4:8, :], ALU.add)
                        nc.vector.tensor_tensor(f4[:, 0:2, :], f4[:, 0:2, :],
                                                f4[:, 2:4, :], ALU.add)
                        nc.vector.tensor_tensor(f1[:, bt, :], f4[:, 0, :],
                                                f4[:, 1, :], ALU.add)
                    # batched over all b_tiles: [128, bts, *]
                    s1a = _ap_with(f1[:], HID, [[PAYW, bts], [1, HID]])
                    s02a = _ap_with(f1[:], 0, [[PAYW, bts], [1, HID]])
                    swa = _ap_with(f1[:], 2 * HID, [[PAYW, bts], [1, H]])
                    den = bhpool.tile([128, bts, H], F32, tag="den")
                    nc.vector.tensor_scalar_add(den[:], swa, EPS_S)
                    dinv = bhpool.tile([128, bts, H], F32, tag="dinv")
                    nc.vector.reciprocal(dinv[:], den[:])
                    rot = bhpool.tile([128, bts, HID], F32, tag="rot")
                    c1b = _ap_with(C1rep[:], 0, [[0, bts], [1, HID]])
                    nc.vector.tensor_tensor(rot[:], s1a, c1b, ALU.mult)
                    tb = bhpool.tile([128, bts, HID], F32, tag="tb")
                    s1swap = _ap_with(f1[:], HID + 1,
                                      [[PAYW, bts], [2, HID // 2], [-1, 2]])
                    c2b = _ap_with(C2rep[side][:], 0, [[0, bts], [1, HID]])
                    nc.vector.tensor_tensor(tb[:], s1swap, c2b, ALU.mult)
                    nc.vector.tensor_tensor(rot[:], rot[:], tb[:], ALU.add)
                    nc.vector.tensor_tensor(rot[:], rot[:], s02a, ALU.add)
                    ret = bhpool.tile([128, bts, HID], F32, tag="ret")
                    dinvb = _ap_with(dinv[:], 0, [[H, bts], [1, H], [0, D]])
                    nc.vector.tensor_tensor(ret[:], rot[:], dinvb, ALU.mult)
                    neg = bhpool.tile([128, bts, HID], F32, tag="neg")
                    nc.vector.tensor_scalar_min(neg[:], ret[:], 0.0)
                    en = bhpool.tile([128, bts, HID], F32, tag="en")
                    nc.scalar.activation(en[:], neg[:], AF.Exp)
                    o_all = outs_all[:, mp, :, :]
                    nc.vector.tensor_scalar_max(ret[:], ret[:], 0.0)
                    nc.vector.tensor_scalar_add(en[:], en[:], -1.0)
                    nc.vector.tensor_tensor(o_all, ret[:], en[:], ALU.add)
                    for bt in range(bts):
                        o = outs_all[:, mp, bt, :]
                        oT_ps = hpspool.tile([HID, 128], F32, space=PSUM, tag="oT_ps")
                        nc.tensor.transpose(oT_ps[:], o, id128[:])
                        oT = hpool.tile([HID, 128], F32, tag="oT")
                        nc.scalar.activation(oT[:], oT_ps[:], AF.Copy)
                        tt = hpspool.tile([128, c.AV], F32, space=PSUM, tag="tt")
                        nc.tensor.matmul(out=tt[:], lhsT=oT[:], rhs=w1sb[:], start=True, stop=True)
                        th = hpool.tile([128, c.AV], F32, tag="th")
                        nc.vector.tensor_tensor(th[:], tt[:], b1rep[:], ALU.add)
                        nc.scalar.activation(th[:], th[:], AF.Tanh)
                        nc.vector.tensor_tensor(th[:], th[:], w2rep[:], ALU.mult)
                        rsum = hpool.tile([128, 1], F32, tag="rsum")
                        nc.vector.tensor_reduce(rsum[:], th[:], mybir.AxisListType.X, ALU.add)
                        sp = hpspool.tile([1, 1], F32, space=PSUM, tag="sp")
                        nc.tensor.matmul(out=sp[:], lhsT=rsum[:], rhs=onescol[:], start=True, stop=True)
                        nc.vector.tensor_tensor(acc4[:, mp:mp + 1], acc4[:, mp:mp + 1], sp[:], ALU.add)

                # software-pipelined emission: gather(ci+1) before compute(ci)
                for mp in range(c.n_mp):
                    emi_sb = ipool.tile([128, 3 * T8], I16, tag="emi_sb")
                    nc.sync.dma_start(
                        emi_sb[:], emi16.ap()[mp * 128:(mp + 1) * 128, :])
                    emi_sbs[mp] = emi_sb
                    tl_sb = tlpool.tile([128, T8], I16, tag="tl_sb")
                    nc.sync.dma_start(tl_sb[:], tl16.ap()[mp * 128:(mp + 1) * 128, :])
                    tl_sbs[mp] = tl_sb
                    for ci in range(len(calls)):
                        emit_gather(mp, ci)
                        if ci > 1:
                            emit_compute_scatter(mp, ci - 2)
                    emit_compute_scatter(mp, len(calls) - 2)
                    emit_compute_scatter(mp, len(calls) - 1)
                    emit_head(mp)

                if c.dbg:
                    dbg_ctx = tc.tile_pool(name="dbgp", bufs=1)
                    dpool_dbg = dbg_ctx.__enter__()
                    for b0 in range(0, c.b_tiles, 2):
                        dsb = dpool_dbg.tile([128, 2, PAYW], F32, tag="dsb",
                                             name=f"dsb{b0}")
                        src = bass.AP(accs[0][:].tensor,
                                      accs[0][:].offset + b0 * 128 * c.acc_step,
                                      [[c.acc_step, 128], [128 * c.acc_step, 2],
                                       [1, PAYW]])
                        nc.sync.dma_start(dsb[:], src)
                        dst = bass.AP(dbg_acc.ap().tensor, b0 * 128 * PAYW,
                                      [[PAYW, 128], [128 * PAYW, 2], [1, PAYW]])
                        nc.sync.dma_start(dst, dsb[:])
                    tsb = dpool_dbg.tile([128, 16, 64], F16, tag="tsb")
                    nc.sync.dma_start(
                        tsb[:], bass.AP(table_t[:].tensor, 0,
                                        [[64, 128], [128 * 64, 16], [1, 64]]))
                    dst2 = bass.AP(dbg_tab.ap().tensor, 0,
                                   [[64, 128], [128 * 64, 16], [1, 64]])
                    nc.sync.dma_start(dst2, tsb[:])
                    esb = dpool_dbg.tile([128, 384], F16, tag="esb")
                    nc.gpsimd.dma_gather(
                        out_ap=esb[:].rearrange("p (a b) -> p a b", a=3),
                        in_ap=table_pairs,
                        idxs_ap=emi_sbs[c.n_mp - 1][:, 0:24],
                        num_idxs=384, num_idxs_reg=384,
                        elem_size=128, single_packet=False)
                    nc.sync.dma_start(dbg_ed.ap(), esb[:])
                    dbg_ctx.__exit__(None, None, None)

                # ---------- semantic softmax + product MLP ----------
                sin_t = dpool.tile([1, 128], F32, tag="sin")
                sout_t = dpool.tile([1, 128], F32, tag="sout")
                zrow = hpool.tile([1, 128], F32, tag="zrow")
                nc.vector.memset(zrow[:], 0.0)
                nc.sync.dma_start(sin_t[:], zrow[:])
                nc.sync.dma_start(sin_t[0:1, 0:c.n_mp], acc4[:])
                nc.gpsimd.collective_compute(
                    "AllReduce", ALU.add,
                    replica_groups=[list(range(c.n_cores))],
                    ins=[sin_t.opt()], outs=[sout_t.opt()],
                )
                s4 = hpool.tile([1, c.n_mp], F32, tag="s4")
                nc.sync.dma_start(s4[:], sout_t[0:1, 0:c.n_mp])
                e4 = hpool.tile([1, c.n_mp], F32, tag="e4")
                nc.scalar.activation(e4[:], s4[:], AF.Exp, scale=1.0 / c.B)
                beta = hpool.tile([1, c.n_mp], F32, tag="beta")
                for sd in range(2):
                    ssum = hpool.tile([1, 1], F32, tag="ssum")
                    nc.vector.tensor_reduce(ssum[:], e4[:, 2 * sd:2 * sd + 2],
                                            mybir.AxisListType.X, ALU.add)
                    sinv = hpool.tile([1, 1], F32, tag="sinv")
                    nc.vector.reciprocal(sinv[:], ssum[:])
                    nc.vector.tensor_scalar_mul(beta[:, 2 * sd:2 * sd + 2],
                                                e4[:, 2 * sd:2 * sd + 2], sinv[:])
                bc_ps = hpspool.tile([128, c.n_mp], F32, space=PSUM, tag="bc_ps")
                nc.tensor.matmul(out=bc_ps[:], lhsT=ones1[:], rhs=beta[:], start=True, stop=True)
                bcol = keep.tile([128, c.n_mp], F32, tag="bcol")
                nc.vector.tensor_copy(bcol[:], bc_ps[:])

                for bt in range(c.b_tiles):
                    hu = hpool.tile([128, HID], F32, tag="hu")
                    hi_ = hpool.tile([128, HID], F32, tag="hi_")
                    t0 = hpool.tile([128, HID], F32, tag="t0")
                    nc.vector.tensor_scalar_mul(hu[:], outs_all[:, 0, bt, :], bcol[:, 0:1])
                    nc.vector.tensor_scalar_mul(t0[:], outs_all[:, 1, bt, :], bcol[:, 1:2])
                    nc.vector.tensor_tensor(hu[:], hu[:], t0[:], ALU.add)
                    nc.vector.tensor_scalar_mul(hi_[:], outs_all[:, 2, bt, :], bcol[:, 2:3])
                    nc.vector.tensor_scalar_mul(t0[:], outs_all[:, 3, bt, :], bcol[:, 3:4])
                    nc.vector.tensor_tensor(hi_[:], hi_[:], t0[:], ALU.add)
                    xx = hpool.tile([128, HID], F32, tag="xx")
                    nc.vector.tensor_tensor(xx[:], hu[:], hi_[:], ALU.mult)
                    xT_ps = hpspool.tile([HID, 128], F32, space=PSUM, tag="xT_ps")
                    nc.tensor.transpose(xT_ps[:], xx[:], id128[:])
                    xT = hpool.tile([HID, 128], F32, tag="xT")
                    nc.vector.tensor_copy(xT[:], xT_ps[:])
                    yy = hpspool.tile([128, c.CH], F32, space=PSUM, tag="yy")
                    nc.tensor.matmul(out=yy[:], lhsT=xT[:], rhs=cw1sb[:], start=True, stop=True)
                    ya = hpool.tile([128, c.CH], F32, tag="ya")
                    nc.vector.tensor_tensor(ya[:], yy[:], CB1rep[:], ALU.add)
                    nc.scalar.activation(ya[:], ya[:], AF.Relu)
                    l0t = hpool.tile([128, c.CH], F32, tag="l0t")
                    nc.vector.tensor_tensor(l0t[:], ya[:], CW20rep[:], ALU.mult)
                    l0 = hpool.tile([128, 1], F32, tag="l0")
                    nc.vector.tensor_reduce(l0[:], l0t[:], mybir.AxisListType.X, ALU.add)
                    nc.vector.tensor_tensor(l0t[:], ya[:], CW21rep[:], ALU.mult)
                    l1 = hpool.tile([128, 1], F32, tag="l1")
                    nc.vector.tensor_reduce(l1[:], l0t[:], mybir.AxisListType.X, ALU.add)
                    dl = hpool.tile([128, 1], F32, tag="dl")
                    ot = hpool.tile([128, 2], F32, tag="ot")
                    nc.vector.tensor_tensor(dl[:], l0[:], l1[:], ALU.subtract)
                    nc.scalar.activation(ot[:, 0:1], dl[:], AF.Sigmoid)
                    nc.vector.tensor_tensor(dl[:], l1[:], l0[:], ALU.subtract)
                    nc.scalar.activation(ot[:, 1:2], dl[:], AF.Sigmoid)
                    nc.sync.dma_start(outd.ap()[bt * 128:(bt + 1) * 128, :], ot[:])

    nc.compile()
    return nc


# ---------------------------------------------------------------------------
# host side: sharding / packing (integer work only)
# ---------------------------------------------------------------------------

def _mp_arrays(inputs, mp):
    if mp < 2:
        return np.asarray(inputs["emi_user"][mp]), np.asarray(inputs["tgt_user"][mp])
    return np.asarray(inputs["emi_item"][mp - 2]), np.asarray(inputs["tgt_item"][mp - 2])


def _rows_of(emi, c: Cfg):
    """Global node id -> padded table row id."""
    return (emi // c.nodes_real) * c.nodes_core + emi % c.nodes_real


def _band_cls(emi, tgt, k, c: Cfg):
    """Per-core (band, cls, tloc, rows) for the selected instances."""
    lo, hi = k * c.B_loc, (k + 1) * c.B_loc
    sel = np.nonzero((tgt >= lo) & (tgt < hi))[0]
    r_all = _rows_of(emi[sel], c)
    t_all = tgt[sel] - lo
    cls = (r_all[:, 0] & 1) + 2 * (r_all[:, 1] & 1) + 4 * (r_all[:, 2] & 1)
    order = np.argsort(t_all, kind="stable")
    ts = t_all[order]
    rk = np.arange(ts.size) - np.searchsorted(ts, ts, side="left")
    band = np.empty_like(rk)
    slot = np.empty_like(rk)
    band[order] = rk // c.S
    slot[order] = rk % c.S
    return band, slot, cls, t_all, r_all


def make_plan(inputs, cfg: Cfg):
    """tiles_bc [n_bands, 8]: tiles per (rank-band, parity-class), maxed over
    every (metapath, core) so one compiled layout serves all shards."""
    c = cfg
    nb = 0
    cnts = []
    for mp in range(c.n_mp):
        emi, tgt = _mp_arrays(inputs, mp)
        for k in range(c.n_cores):
            band, slot, cls, t_all, _ = _band_cls(emi, tgt, k, c)
            nb = max(nb, int(band.max()) + 1)
            cnt = np.zeros((int(band.max()) + 1, 8), np.int64)
            np.add.at(cnt, (band, cls), 1)
            cnts.append(cnt)
    tiles_bc = np.zeros((nb, 8), np.int64)
    for cnt in cnts:
        t = (cnt + 127) // 128
        tiles_bc[:t.shape[0]] = np.maximum(tiles_bc[:t.shape[0]], t)
    return tiles_bc, int(tiles_bc.sum())


def _wrap16(vals):
    """[N] values (N % 16 == 0) -> [128, N/16] int16, q7 wrapped layout."""
    v = np.asarray(vals).astype(np.int16).reshape(-1, 16)
    return np.ascontiguousarray(np.tile(v.T, (8, 1)))


def _pack_metapath(emi, tgt, k, c: Cfg):
    """Pack one (metapath, core) shard: band-major, class-sorted in band.

    Row index for the scatter is tgt*S + rank%S (unique within any call,
    since calls never span a band boundary); padding goes to the trash row.
    Returns (emi16 [128, sum(3*nt*8)], tl16 [128, T*8])."""
    band, slot, cls, t_all, r_all = _band_cls(emi, tgt, k, c)
    tiles_bc = c.tiles_bc
    E = c.T * 128
    r_sh = np.zeros((E, 3), np.int64)
    rowi = np.full((E,), c.trash_row, np.int64)
    tpos = 0
    for b in range(tiles_bc.shape[0]):
        for cl in range(8):
            ntiles = int(tiles_bc[b][cl])
            if ntiles == 0:
                continue
            seg = np.nonzero((band == b) & (cls == cl))[0]
            assert seg.size <= ntiles * 128, (b, cl, seg.size, ntiles)
            base = tpos * 128
            r_sh[base:base + seg.size] = r_all[seg]
            dummy = np.array([(cl >> l) & 1 for l in range(3)], np.int64)
            r_sh[base + seg.size:base + ntiles * 128] = dummy
            rowi[base:base + seg.size] = t_all[seg] * c.S + slot[seg]
            tpos += ntiles
    assert tpos == c.T
    pair = r_sh >> 1          # [E, 3] pair-row gather indices (< 20480)
    emi_calls, tl_calls = [], []
    t0 = 0
    for (_, _, nt) in c.calls:
        blk = slice(t0 * 128, (t0 + nt) * 128)
        stream = np.concatenate([pair[blk, l] for l in range(3)])
        emi_calls.append(_wrap16(stream))
        tl_calls.append(_wrap16(rowi[blk]))
        t0 += nt
    assert t0 == c.T
    return (np.concatenate(emi_calls, axis=1),
            np.concatenate(tl_calls, axis=1))


def prepare(inputs, cfg: Cfg):
    c = cfg
    tbc, T = make_plan(inputs, cfg)
    c.tiles_bc = tbc
    c.T = T

    f0, f1 = np.asarray(inputs["feats0"]), np.asarray(inputs["feats1"])
    feats_all = np.concatenate([f0, f1], axis=0)
    attn4 = np.stack([np.asarray(inputs["attn_user"][p]).reshape(-1) for p in range(2)] +
                     [np.asarray(inputs["attn_item"][p]).reshape(-1) for p in range(2)])
    rv = np.asarray(inputs["r_vec"])[0].reshape(-1).astype(np.float32)

    in_maps = []
    for k in range(c.n_cores):
        m = {}
        lo_n = k * c.nodes_real
        fs = feats_all[lo_n:lo_n + c.nodes_real]
        pad = c.nodes_core - c.nodes_real
        if pad:
            fs = np.concatenate([fs, np.zeros((pad, c.F0), np.float32)], axis=0)
        m["feats"] = np.ascontiguousarray(fs, np.float32)
        tw = "0" if lo_n < f0.shape[0] else "1"
        vals = {
            **{nm: np.asarray(inputs[f"tower{tw}_{nm}"], np.float32)
               for nm in ("pw", "pb", "w2", "b2", "g", "be")},
            "rvec": rv, "attn": attn4.astype(np.float32),
            "suw1": inputs["su_w1"], "sub1": inputs["su_b1"],
            "suw2": inputs["su_w2"], "siw1": inputs["si_w1"],
            "sib1": inputs["si_b1"], "siw2": inputs["si_w2"],
            "cw1": inputs["cw1"], "cb1": inputs["cb1"],
            "cw2t": np.asarray(inputs["cw2"], np.float32).T,
        }
        blob = np.zeros(WB_TOTAL, np.float32)
        for nm, sz in _WB_FIELDS:
            v = np.asarray(vals[nm], np.float32).reshape(-1)
            assert v.size == sz, (nm, v.size, sz)
            blob[WB_OFF[nm]:WB_OFF[nm] + sz] = v
        m["wblob"] = blob
        emi_l, tl_l = [], []
        for mp in range(c.n_mp):
            emi, tgt = _mp_arrays(inputs, mp)
            e16, t16 = _pack_metapath(emi, tgt, k, c)
            emi_l.append(e16)
            tl_l.append(t16)
        m["emi16"] = np.concatenate(emi_l, axis=0)
        m["tl16"] = np.concatenate(tl_l, axis=0)
        in_maps.append(m)
    return in_maps


# ---------------------------------------------------------------------------
# PJRT SPMD runner (axon path)
# ---------------------------------------------------------------------------


class SpmdRunner:
    def __init__(self, nc, n_cores: int):
        import jax
        from jax.sharding import Mesh, PartitionSpec, NamedSharding
        from jax.experimental.shard_map import shard_map
        from concourse.bass2jax import (
            _bass_exec_p, install_neuronx_cc_hook, partition_id_tensor)

        self.jax = jax
        install_neuronx_cc_hook()
        self.nc = nc
        self.n_cores = n_cores
        partition_name = nc.partition_id_tensor.name if nc.partition_id_tensor else None
        in_names, out_names, out_avals, zero_outs = [], [], [], []
        for alloc in nc.m.functions[0].allocations:
            if not isinstance(alloc, mybir.MemoryLocationSet):
                continue
            name = alloc.memorylocations[0].name
            if alloc.kind == "ExternalInput":
                if name != partition_name:
                    in_names.append(name)
            elif alloc.kind == "ExternalOutput":
                out_names.append(name)
                shape = tuple(alloc.tensor_shape)
                dtype = mybir.dt.np(alloc.dtype)
                out_avals.append(jax.core.ShapedArray(shape, dtype))
                zero_outs.append(np.zeros(shape, dtype))
        self.dbg_name = nc.dbg_addr.name if nc.dbg_addr is not None else None
        n_params = len(in_names)
        in_names = in_names + out_names
        if partition_name is not None:
            in_names.append(partition_name)
        self.in_names, self.out_names = in_names, out_names
        self.n_params, self.out_avals, self.zero_outs = n_params, out_avals, zero_outs

        def _body(*args):
            operands = list(args)
            if partition_name is not None:
                operands.append(partition_id_tensor())
            outs = _bass_exec_p.bind(
                *operands,
                out_avals=tuple(out_avals),
                in_names=tuple(in_names),
                out_names=tuple(out_names),
                lowering_input_output_aliases=(),
                sim_require_finite=True,
                sim_require_nnan=True,
                nc=nc,
            )
            return tuple(outs)

        devices = jax.devices()[:n_cores]
        assert len(devices) == n_cores
        self.mesh = Mesh(np.asarray(devices), ("core",))
        donate = tuple(range(n_params, n_params + len(out_names)))
        in_specs = (PartitionSpec("core"),) * (n_params + len(out_names))
        out_specs = (PartitionSpec("core"),) * len(out_names)
        self.sharded = jax.jit(
            shard_map(_body, mesh=self.mesh, in_specs=in_specs,
                      out_specs=out_specs, check_rep=False),
            donate_argnums=donate, keep_unused=True)
        self.sharding = NamedSharding(self.mesh, PartitionSpec("core"))

    def stage_inputs(self, in_maps):
        jax = self.jax
        if self.dbg_name is not None:
            in_maps = [{**m, self.dbg_name: np.zeros((1, 2), np.uint32)}
                       for m in in_maps]
        staged = []
        for i in range(self.n_params):
            name = self.in_names[i]
            arr = np.concatenate([np.asarray(m[name]) for m in in_maps], axis=0)
            staged.append(jax.device_put(arr, self.sharding))
        jax.block_until_ready(staged)
        self.staged = staged

    def _zeros(self):
        jax = self.jax
        zs = [jax.device_put(
            np.zeros((self.n_cores * z.shape[0], *z.shape[1:]), z.dtype),
            self.sharding) for z in self.zero_outs]
        jax.block_until_ready(zs)
        return zs

    def run(self):
        jax = self.jax
        outs = self.sharded(*self.staged, *self._zeros())
        jax.block_until_ready(outs)
        return [
            {name: np.asarray(outs[i]).reshape(self.n_cores, *self.out_avals[i].shape)[k]
             for i, name in enumerate(self.out_names)}
            for k in range(self.n_cores)
        ]

    def bench(self, iters=20, warmup=3):
        import time
        jax = self.jax
        times = []
        for it in range(warmup + iters):
            zs = self._zeros()
            t0 = time.perf_counter()
            outs = self.sharded(*self.staged, *zs)
            jax.block_until_ready(outs)
            dt = time.perf_counter() - t0
            if it >= warmup:
                times.append(dt)
            del outs
        times = np.array(times)
        return {"min_s": float(times.min()), "med_s": float(np.median(times)),
                "mean_s": float(times.mean()), "n": iters}


_CACHE = {}


def kernel(**inputs) -> np.ndarray:
    cfg = Cfg()
    in_maps = prepare(inputs, cfg)
    key = (cfg.T, cfg.tiles_bc.tobytes())
    if key not in _CACHE:
        nc = build_program(cfg)
        _CACHE[key] = (nc, SpmdRunner(nc, cfg.n_cores))
    nc, runner = _CACHE[key]
    runner.stage_inputs(in_maps)
    res = runner.run()
    out = np.empty((cfg.B, 2), np.float32)
    for k in range(cfg.n_cores):
        out[k * cfg.B_loc:(k + 1) * cfg.B_loc] = res[k]["out"]
    return out
